# revision 31
# baseline (speedup 1.0000x reference)
"""BERT-base (12-layer) forward pass on 8 Trainium2 NeuronCores.

Strategy: data-parallel over batch (B=8 -> 1 sequence per core), no
collectives. Host casts weights to bf16 and folds each LayerNorm's gain
into the consuming weight matrices (bias folded into effective biases),
so on-device LayerNorm only produces z = (x - mean) * rstd.

Key scheduling ideas (vs the straightforward version):
- S-split software pipelining: AO/FFN1/FFN2/QK matmul groups process the
  sequence in two halves of 256, so each LayerNorm's serial stats chain
  for half A runs concurrently with matmuls of half B -> the Tensor
  engine never drains (keeps the PE HAM clock at 2.4 GHz).
- LN stats are produced in broadcast form directly: mean/meansq rows are
  computed with M=128 all-ones stationary matmuls, eliminating the
  GpSimd partition-broadcast and [1,S] row ops from the critical chain.
- Residual biases (bao + b_prev, bio + b_ln1) enter PSUM via K=1
  rank-1 matmuls appended to each accumulation group; the residual add
  is then a single scalar_tensor_tensor (g_prev * z_prev + psum).
- Activation-table swaps (exp/gelu/abs_rsqrt) are prefetched with dummy
  1-element activations during matmul phases, off the critical path.
- Paired-head softmax: both heads of a 128-feature block share one
  [P,1024] PSUM tile and a single fused Exp activation.
"""
import sys
import os

if "/opt/trn_rl_repo" not in sys.path:
    sys.path.insert(0, "/opt/trn_rl_repo")

import numpy as np
import ml_dtypes

import concourse.bass as bass
from concourse import bacc
import concourse.tile as tile
from concourse import mybir
from concourse.bass_utils import run_bass_kernel_spmd
from concourse.masks import make_identity

F32 = mybir.dt.float32
F32R = mybir.dt.float32r
BF16 = mybir.dt.bfloat16
FP16 = mybir.dt.float16
INT32 = mybir.dt.int32
AF = mybir.ActivationFunctionType
ALU = mybir.AluOpType

# Model dims (hardcoded per problem spec)
B, S, H, NH, L, F = 8, 512, 768, 12, 12, 3072
V, TV, PP = 21128, 2, 512
DH = H // NH            # 64
P = 128
HT = H // P             # 6
FT = F // P             # 24
ST = S // P             # 4
SH = S // 2             # 256  (sequence half)
EPS = 1e-12
NCORES = 8

NL = int(os.environ.get("BERT_KERNEL_LAYERS", str(L)))

_CACHE = {}


def _build():
    nc = bacc.Bacc("TRN2", target_bir_lowering=False, debug=False)

    # ---- DRAM I/O ----
    WQKVO = nc.dram_tensor("wqkvo", [NL, 4, P, HT, H], FP16, kind="ExternalInput")
    WI = nc.dram_tensor("wi", [NL, P, HT, F], FP16, kind="ExternalInput")
    WIO = nc.dram_tensor("wio", [NL, P, FT, H], FP16, kind="ExternalInput")
    PARAMS = nc.dram_tensor("params", [NL, P, 76], F32, kind="ExternalInput")
    CR = nc.dram_tensor("cr", [NL, 1, 2 * H], FP16, kind="ExternalInput")
    BVREP = nc.dram_tensor("bvrep", [NL, P, H], F32, kind="ExternalInput")
    TOK = nc.dram_tensor("tok", [V, H], F32, kind="ExternalInput")
    POSN = nc.dram_tensor("posn", [P, ST, H], F32, kind="ExternalInput")
    DTE = nc.dram_tensor("dte", [P, H], F32, kind="ExternalInput")
    IDS = nc.dram_tensor("ids", [P, ST], INT32, kind="ExternalInput")
    SEGF = nc.dram_tensor("segf", [P, ST], F32, kind="ExternalInput")
    MASKT = nc.dram_tensor("maskt", [P, ST], F32, kind="ExternalInput")
    OUT = nc.dram_tensor("out", [H, S], F32, kind="ExternalOutput")

    outv = OUT[:].rearrange("(ht p) s -> p ht s", p=P)

    DEBUG_TAPS = int(os.environ.get("BERT_DEBUG_TAPS", "0"))
    TAPS = ["z0", "q", "k", "ctx", "pre1", "z1", "pre2"]
    if DEBUG_TAPS:
        DBG = nc.dram_tensor("dbg", [len(TAPS), P, HT, S], F32,
                             kind="ExternalOutput")

    def hs(h):
        return slice(h * SH, (h + 1) * SH)

    from contextlib import ExitStack

    with tile.TileContext(nc) as tc:
        with ExitStack() as es:
            ec = es.enter_context
            cpool = ec(tc.tile_pool(name="const", bufs=1))
            apool = ec(tc.tile_pool(name="act", bufs=1))     # big per-layer activations
            rpool = ec(tc.tile_pool(name="res", bufs=2))     # z (bf16) ping-pong
            crpool = ec(tc.tile_pool(name="crp", bufs=2))    # bias rows per layer
            scr = ec(tc.tile_pool(name="scr", bufs=6))       # [P,256] scratch
            t1p = ec(tc.tile_pool(name="t1p", bufs=7))       # LN t1 tiles
            rsdpool = ec(tc.tile_pool(name="rsd", bufs=2))   # rstd|mq per half
            rows = ec(tc.tile_pool(name="rows", bufs=2))     # [1,S] rows
            bcp = ec(tc.tile_pool(name="bcast", bufs=4))     # attn bcast temps
            bvpool = ec(tc.tile_pool(name="bv", bufs=1))
            eppool = ec(tc.tile_pool(name="ep", bufs=2))     # exp tiles
            pp = ec(tc.tile_pool(name="pp", bufs=2, space="PSUM"))    # 2 banks
            psc = ec(tc.tile_pool(name="psc", bufs=2, space="PSUM"))  # 4 banks
            pcx = ec(tc.tile_pool(name="pcx", bufs=2, space="PSUM"))  # 2 banks
            # ---- persistent constants ----
            params = cpool.tile([P, NL, 76], F32, tag="params")
            nc.sync.dma_start(params[:], PARAMS[:].rearrange("l p c -> p l c"))
            maskt = cpool.tile([P, ST], F32, tag="maskt")
            nc.sync.dma_start(maskt[:], MASKT[:])
            ones_b = cpool.tile([P, P], BF16, tag="ones")
            nc.any.memset(ones_b[:], 1.0)
            ones_bh = cpool.tile([P, P], BF16, tag="onesbh")
            nc.any.memset(ones_bh[:], 1.0 / H)
            onesrow = cpool.tile([1, S], FP16, tag="onesrow")
            nc.any.memset(onesrow[:], 1.0)
            onesf = cpool.tile([1, P], F32, tag="onesf")
            nc.any.memset(onesf[:], 1.0)
            epsc = cpool.tile([P, 1], F32, tag="epsc")
            nc.any.memset(epsc[:], EPS)
            dscr = cpool.tile([1, 1], F32, tag="dscr")
            nc.any.memset(dscr[:], 0.25)

            # ---- persistent activations ----
            qTb = apool.tile([P, HT, S], FP16, tag="qTb")
            kTb = apool.tile([P, HT, S], FP16, tag="kTb")
            ctxTb = apool.tile([P, HT, S], FP16, tag="ctxTb")
            v_aug = apool.tile([P, ST, NH * (DH + 1)], BF16, tag="v_aug")
            hTb = apool.tile([P, FT, S], FP16, tag="hTb")
            preF = apool.tile([P, HT, S], FP16, tag="preF")

            # ones columns of v_aug (written once; per-layer V writes leave them)
            va_view = v_aug[:].rearrange("p st (h d) -> p st h d", d=DH + 1)
            nc.any.memset(va_view[:, :, :, DH : DH + 1], 1.0)

            # ============ embedding (scoped pool, released after) ============
            xzb = rpool.tile([P, HT, S], FP16, tag="resz")
            with tc.tile_pool(name="embp", bufs=3) as embp:
                posn = embp.tile([P, ST, H], F32, tag="posn", bufs=1)
                nc.sync.dma_start(posn[:], POSN[:])
                dte = embp.tile([P, H], F32, tag="dte", bufs=1)
                nc.sync.dma_start(dte[:], DTE[:])
                ids = embp.tile([P, ST], INT32, tag="ids", bufs=1)
                nc.sync.dma_start(ids[:], IDS[:])
                segf = embp.tile([P, ST], F32, tag="segf", bufs=1)
                nc.sync.dma_start(segf[:], SEGF[:])
                ident = embp.tile([P, P], F32, tag="ident", bufs=1)
                make_identity(nc, ident)

                for st in range(ST):
                    x0 = embp.tile([P, H], F32, tag="x0")
                    nc.gpsimd.indirect_dma_start(
                        out=x0[:],
                        out_offset=None,
                        in_=TOK[:],
                        in_offset=bass.IndirectOffsetOnAxis(
                            ap=ids[:, st : st + 1], axis=0
                        ),
                    )
                    # + (pos + type0) + seg*(type1-type0)
                    tseg = embp.tile([P, H], F32, tag="tseg")
                    nc.scalar.activation(
                        tseg[:], dte[:], AF.Copy, scale=segf[:, st : st + 1]
                    )
                    nc.vector.tensor_add(out=x0[:], in0=x0[:], in1=posn[:, st])
                    nc.gpsimd.tensor_tensor(x0[:], x0[:], tseg[:], ALU.add)
                    # LayerNorm along free dim (features); z only (g,b folded)
                    s1 = embp.tile([P, 1], F32, tag="s1")
                    nc.vector.reduce_sum(s1[:], x0[:], axis=mybir.AxisListType.X)
                    sqs = embp.tile([P, H], F32, tag="sqs")
                    ssq = embp.tile([P, 1], F32, tag="ssq")
                    nc.scalar.activation(sqs[:], x0[:], AF.Square, accum_out=ssq[:])
                    mean = embp.tile([P, 1], F32, tag="mean")
                    nc.any.tensor_scalar_mul(mean[:], s1[:], 1.0 / H)
                    msq = embp.tile([P, 1], F32, tag="msq")
                    nc.any.tensor_scalar_mul(msq[:], ssq[:], 1.0 / H)
                    var = embp.tile([P, 1], F32, tag="var")
                    nc.vector.tensor_tensor(var[:], mean[:], mean[:], ALU.mult)
                    nc.vector.tensor_tensor(var[:], msq[:], var[:], ALU.subtract)
                    rstd = embp.tile([P, 1], F32, tag="rstd")
                    lnv0 = embp.tile([P, 1], F32, tag="lnv0")
                    nc.scalar.activation(lnv0[:], var[:], AF.Ln, bias=epsc[:])
                    nc.scalar.activation(rstd[:], lnv0[:], AF.Exp, scale=-0.5)
                    negmr = embp.tile([P, 1], F32, tag="negmr")
                    nc.vector.tensor_tensor(negmr[:], mean[:], rstd[:], ALU.mult)
                    nc.any.tensor_scalar_mul(negmr[:], negmr[:], -1.0)
                    nc.scalar.activation(
                        x0[:], x0[:], AF.Identity, bias=negmr[:], scale=rstd[:]
                    )
                    # transpose into z layout (bf16)
                    for ht in range(HT):
                        pt = pp.tile([P, S], F32, tag="proj")
                        nc.tensor.transpose(
                            pt[:, :P], x0[:, P * ht : P * (ht + 1)], ident[:]
                        )
                        nc.scalar.copy(xzb[:, ht, P * st : P * (st + 1)], pt[:, :P])

            # ============ transformer layers ============

            def ln_half(pst, h, srcF, l, final):
                """Stats chain + apply for one sequence half.

                pst[:, :SH] = broadcast mean, pst[:, SH:] = broadcast meansq.
                Writes zdst bf16 (or the final DRAM output when `final`).
                Returns the z tile (None when final)."""
                pb = params[:, l, :]
                m2 = scr.tile([P, SH], F32, tag="s")
                nc.scalar.activation(m2[:], pst[:, 0:SH], AF.Square)
                varb = scr.tile([P, SH], F32, tag="s")
                nc.vector.tensor_tensor(varb[:], pst[:, SH:S], m2[:], ALU.subtract)
                rsd = rsdpool.tile([P, S], FP16, tag="rsd")
                nc.scalar.activation(
                    rsd[:, 0:SH], varb[:], AF.Abs_reciprocal_sqrt, bias=epsc[:]
                )
                return rsd

            def ln_sub_mean(pst, h, srcF, t1s):
                # phase A: t1 = x - mean_b (Vector: PSUM read); starts
                # before rstd is ready
                for kt in range(HT):
                    t1 = t1p.tile([P, SH], FP16, tag="t")
                    nc.vector.tensor_tensor(
                        t1[:], srcF[:, kt, hs(h)], pst[:, 0:SH], ALU.subtract
                    )
                    t1s.append(t1)

            def ln_apply(rsd, h, t1s, zdst, l, final):
                pb = params[:, l, :]
                for kt in range(HT):
                    eng = nc.vector
                    if not final:
                        eng.tensor_tensor(
                            zdst[:, kt, hs(h)], t1s[kt][:], rsd[:, 0:SH],
                            ALU.mult,
                        )
                    else:
                        of = scr.tile([P, SH], F32, tag="s")
                        eng.tensor_tensor(of[:], t1s[kt][:], rsd[:, 0:SH],
                                          ALU.mult)
                        nc.scalar.activation(
                            of[:], of[:], AF.Identity,
                            bias=pb[:, 30 + kt : 31 + kt],
                            scale=pb[:, 24 + kt : 25 + kt],
                        )
                        nc.sync.dma_start(outv[:, kt, hs(h)], of[:])

            def tap(name, src, l):
                if not DEBUG_TAPS or l != 0:
                    return
                ti = TAPS.index(name)
                for kt in range(HT):
                    if src.dtype == F32:
                        nc.sync.dma_start(DBG[ti][:, kt], src[:, kt])
                    else:
                        dc = scr.tile([P, S], F32, tag="dbg", bufs=2)
                        nc.scalar.activation(dc[:], src[:, kt], AF.Identity)
                        nc.sync.dma_start(DBG[ti][:, kt], dc[:])

            tap("z0", xzb, 0)

            wqpool = ec(tc.tile_pool(name="wq", bufs=2))
            wfpool = ec(tc.tile_pool(name="wf", bufs=2))
            wopool = ec(tc.tile_pool(name="wo", bufs=1))
            if True:
                for l in range(NL):
                    pb = params[:, l, :]
                    crows = crpool.tile([1, 2 * H], FP16, tag="crows")
                    nc.sync.dma_start(crows[:], CR[l])

                    # ---- Q, K projections (transposed out; consume xzb) ----
                    scope_qk = nc.named_scope(f"qk_{l}"); scope_qk.__enter__()
                    wqk = []
                    for pi in range(2):
                        w = wqpool.tile([P, HT, H], FP16, tag="wqk",
                                        name=f"wqk{pi}")
                        nc.sync.dma_start(w[:], WQKVO[l, pi])
                        wqk.append(w)
                    for h in range(2):
                        for pi, dst in ((0, qTb), (1, kTb)):
                            w = wqk[pi]
                            for nt in range(HT):
                                ps = pp.tile([P, S], F32, tag="proj")
                                for kt in range(HT):
                                    nc.tensor.matmul(
                                        ps[:, :SH],
                                        w[:, kt, P * nt : P * (nt + 1)],
                                        xzb[:, kt, hs(h)],
                                        start=(kt == 0), stop=(kt == HT - 1),
                                    )
                                c = 6 * pi + nt
                                nc.vector.tensor_scalar_add(
                                    dst[:, nt, hs(h)], ps[:, :SH],
                                    pb[:, c : c + 1],
                                )

                    scope_qk.__exit__(None, None, None)
                    tap("q", qTb, l)
                    tap("k", kTb, l)
                    scope_v = nc.named_scope(f"v_{l}"); scope_v.__enter__()
                    # ---- V projection (normal layout, augmented buffer) ----
                    nc.scalar.activation(dscr[:], qTb[0:1, 0, 0:1], AF.Exp)
                    wv = wqpool.tile([P, HT, H], FP16, tag="wqk")
                    nc.sync.dma_start(wv[:], WQKVO[l, 2])
                    bvr = bvpool.tile([P, H], F32, tag="bvr")
                    nc.sync.dma_start(bvr[:], BVREP[l])
                    for st in range(ST):
                        for half in range(2):
                            ps = pp.tile([P, S], F32, tag="proj")
                            for kt in range(HT):
                                nc.tensor.matmul(
                                    ps[:, :384],
                                    xzb[:, kt, P * st : P * (st + 1)],
                                    wv[:, kt, 384 * half : 384 * (half + 1)],
                                    start=(kt == 0), stop=(kt == HT - 1),
                                )
                            dst3 = va_view[:, st, 6 * half : 6 * (half + 1), 0:DH]
                            src3 = ps[:, :384].rearrange("p (h d) -> p h d", d=DH)
                            bv3 = bvr[:, 384 * half : 384 * (half + 1)].rearrange(
                                "p (h d) -> p h d", d=DH
                            )
                            nc.vector.tensor_tensor(dst3, src3, bv3, ALU.add)

                    scope_v.__exit__(None, None, None)
                    scope_at = nc.named_scope(f"attn_{l}"); scope_at.__enter__()
                    # ---- attention (head pairs share ht; fused pair Exp) ----
                    for ht in range(HT):
                        exq = eppool.tile([P, ST, 2 * S], BF16, tag="exq")
                        for kti in range(ST):
                            pshh = psc.tile([P, 2 * S], F32, tag="sc")
                            for sub in range(2):
                                base = 64 * sub
                                nc.tensor.matmul(
                                    pshh[:, S * sub : S * (sub + 1)],
                                    kTb[base : base + DH, ht, P * kti : P * (kti + 1)],
                                    qTb[base : base + DH, ht, :],
                                    start=True, stop=True,
                                )
                            nc.scalar.activation(
                                exq[:, kti], pshh[:], AF.Exp,
                                scale=0.125, bias=maskt[:, kti : kti + 1],
                            )
                        for sub in range(2):
                            hh = 2 * ht + sub
                            base = 64 * sub
                            ppc = pcx.tile([DH + 1, S], F32, tag="cx")
                            for kti in range(ST):
                                nc.tensor.matmul(
                                    ppc[:],
                                    v_aug[:, kti, (DH + 1) * hh : (DH + 1) * (hh + 1)],
                                    exq[:, kti, S * sub : S * (sub + 1)],
                                    start=(kti == 0), stop=(kti == ST - 1),
                                )
                            # 1/denominator on the PSUM row directly, then
                            # broadcast via K=1 matmul on the (idle) PE
                            srow0 = rows.tile([1, S], F32, tag="srow0")
                            nc.vector.tensor_copy(srow0[:], ppc[DH : DH + 1, :])
                            rec = rows.tile([1, S], F32, tag="srow0")
                            nc.vector.reciprocal_approx_fast(rec[:], srow0[:])
                            rb = bcp.tile([DH, S], F32, tag="b")
                            nc.gpsimd.partition_broadcast(rb[:], rec[:])
                            nc.vector.tensor_tensor(
                                ctxTb[base : base + DH, ht, :], ppc[:DH, :],
                                rb[:], ALU.mult,
                            )

                    scope_at.__exit__(None, None, None)
                    tap("ctx", ctxTb, l)
                    scope_ao = nc.named_scope(f"ao_{l}"); scope_ao.__enter__()
                    # ---- attention output + residual + LN1 (S-split) ----
                    wao = wqpool.tile([P, HT, H], FP16, tag="wqk")
                    nc.sync.dma_start(wao[:], WQKVO[l, 3])
                    azb = rpool.tile([P, HT, S], FP16, tag="resz")
                    for h in range(2):
                        pst = pcx.tile([P, S], F32, tag="cx")
                        for nt in range(HT):
                            ps = pp.tile([P, S], F32, tag="proj")
                            for kt in range(HT):
                                nc.tensor.matmul(
                                    ps[:, :SH],
                                    wao[:, kt, P * nt : P * (nt + 1)],
                                    ctxTb[:, kt, hs(h)],
                                    start=(kt == 0), stop=False,
                                )
                            # + (bao + b_prev) via rank-1 K=1 matmul
                            nc.tensor.matmul(
                                ps[:, :SH],
                                crows[0:1, P * nt : P * (nt + 1)],
                                onesrow[0:1, :SH],
                                start=False, stop=True,
                            )
                            # pre1 = ps + g_prev * z_prev
                            nc.vector.scalar_tensor_tensor(
                                preF[:, nt, hs(h)], xzb[:, nt, hs(h)],
                                pb[:, 12 + nt : 13 + nt], ps[:, :SH],
                                ALU.mult, ALU.add,
                            )
                            if h == 0 and nt == 0:
                                # dep-pinned abs_rsqrt table prefetch
                                nc.scalar.activation(
                                    dscr[:], preF[0:1, 0, 0:1],
                                    AF.Abs_reciprocal_sqrt,
                                )
                            # stats (broadcast form)
                            # single accumulation group per bank:
                            # sc2 = [x/H | x*x/H] -> [mean_b | msq_b]
                            sc2 = scr.tile([P, S], BF16, tag="s")
                            nc.scalar.activation(
                                sc2[:, 0:SH], preF[:, nt, hs(h)], AF.Copy,
                                scale=1.0 / H,
                            )
                            nc.vector.tensor_tensor(
                                sc2[:, SH:S], sc2[:, 0:SH],
                                preF[:, nt, hs(h)], ALU.mult,
                            )
                            nc.tensor.matmul(
                                pst[:, 0:S], ones_b[:], sc2[:],
                                start=(nt == 0), stop=(nt == HT - 1),
                            )
                        t1s = []
                        ln_sub_mean(pst, h, preF, t1s)
                        rsd = ln_half(pst, h, preF, l, False)
                        ln_apply(rsd, h, t1s, azb, l, False)
                    tap("pre1", preF, l)
                    tap("z1", azb, l)
                    scope_ao.__exit__(None, None, None)

                    scope_f1 = nc.named_scope(f"ffn1_{l}"); scope_f1.__enter__()
                    # ---- FFN intermediate (gelu) ----
                    for quarter in range(4):
                        wih = wfpool.tile([P, HT, F // 4], FP16, tag="wi")
                        nc.sync.dma_start(
                            wih[:],
                            WI[l][:, :, (F // 4) * quarter : (F // 4) * (quarter + 1)],
                        )
                        for h in range(2):
                            for ntl in range(6):
                                nt = 6 * quarter + ntl
                                ps = pp.tile([P, S], F32, tag="proj")
                                for kt in range(HT):
                                    nc.tensor.matmul(
                                        ps[:, :SH],
                                        wih[:, kt, P * ntl : P * (ntl + 1)],
                                        azb[:, kt, hs(h)],
                                        start=(kt == 0), stop=(kt == HT - 1),
                                    )
                                nc.scalar.activation(
                                    hTb[:, nt, hs(h)], ps[:, :SH], AF.Gelu,
                                    bias=pb[:, 48 + nt : 49 + nt],
                                )

                    scope_f1.__exit__(None, None, None)
                    scope_f2 = nc.named_scope(f"ffn2_{l}"); scope_f2.__enter__()
                    # ---- FFN output + residual + LN2 (S-split) ----
                    wio = wopool.tile([P, FT, H], FP16, tag="wio")
                    nc.sync.dma_start(wio[:], WIO[l])
                    final = l == NL - 1
                    if not final:
                        xzb = rpool.tile([P, HT, S], FP16, tag="resz")
                    for h in range(2):
                        pst = pcx.tile([P, S], F32, tag="cx")
                        for nt in range(HT):
                            ps = pp.tile([P, S], F32, tag="proj")
                            for kt in range(FT):
                                nc.tensor.matmul(
                                    ps[:, :SH],
                                    wio[:, kt, P * nt : P * (nt + 1)],
                                    hTb[:, kt, hs(h)],
                                    start=(kt == 0), stop=False,
                                )
                            # + (bio + b_ln1) via rank-1 K=1 matmul
                            nc.tensor.matmul(
                                ps[:, :SH],
                                crows[0:1, H + P * nt : H + P * (nt + 1)],
                                onesrow[0:1, :SH],
                                start=False, stop=True,
                            )
                            # pre2 = ps + g_ln1 * z_attn
                            nc.vector.scalar_tensor_tensor(
                                preF[:, nt, hs(h)], azb[:, nt, hs(h)],
                                pb[:, 18 + nt : 19 + nt], ps[:, :SH],
                                ALU.mult, ALU.add,
                            )
                            if h == 0 and nt == 0:
                                nc.scalar.activation(
                                    dscr[:], preF[0:1, 0, 0:1],
                                    AF.Abs_reciprocal_sqrt,
                                )
                            # single accumulation group per bank:
                            # sc2 = [x/H | x*x/H] -> [mean_b | msq_b]
                            sc2 = scr.tile([P, S], BF16, tag="s")
                            nc.scalar.activation(
                                sc2[:, 0:SH], preF[:, nt, hs(h)], AF.Copy,
                                scale=1.0 / H,
                            )
                            nc.vector.tensor_tensor(
                                sc2[:, SH:S], sc2[:, 0:SH],
                                preF[:, nt, hs(h)], ALU.mult,
                            )
                            nc.tensor.matmul(
                                pst[:, 0:S], ones_b[:], sc2[:],
                                start=(nt == 0), stop=(nt == HT - 1),
                            )
                        t1s = []
                        ln_sub_mean(pst, h, preF, t1s)
                        rsd = ln_half(pst, h, preF, l, final)
                        ln_apply(rsd, h, t1s, None if final else xzb, l, final)
                    tap("pre2", preF, l)
                    scope_f2.__exit__(None, None, None)

    nc.compile()
    return nc


def _r6(v):
    return np.ascontiguousarray(v.reshape(6, P).T)


def _prep_shared(inputs):
    bf = ml_dtypes.bfloat16
    fh = np.float16
    f32 = np.float32
    emb_g = np.asarray(inputs["emb_g"], f32)
    emb_b = np.asarray(inputs["emb_b"], f32)
    ln1_g = np.asarray(inputs["ln1_g"], f32)
    ln1_b = np.asarray(inputs["ln1_b"], f32)
    ln2_g = np.asarray(inputs["ln2_g"], f32)
    ln2_b = np.asarray(inputs["ln2_b"], f32)

    wqkvo = np.empty((NL, 4, P, HT, H), dtype=fh)
    wi = np.empty((NL, P, HT, F), dtype=fh)
    wio = np.empty((NL, P, FT, H), dtype=fh)
    params = np.zeros((NL, P, 76), dtype=f32)
    cr = np.zeros((NL, 1, 2 * H), dtype=fh)
    bvrep = np.empty((NL, P, H), dtype=f32)

    def tposed(w, kt, dim_out):
        # [H, dim_out] -> [P, kt, dim_out]
        return w.reshape(kt, P, dim_out).transpose(1, 0, 2)

    for l in range(NL):
        gprev = emb_g if l == 0 else ln2_g[l - 1]
        bprev = emb_b if l == 0 else ln2_b[l - 1]
        for pi, name, bname in ((0, "Wq", "bq"), (1, "Wk", "bk"), (2, "Wv", "bv")):
            w = np.asarray(inputs[name][l], f32)
            b = np.asarray(inputs[bname][l], f32)
            wf = w * gprev[:, None]
            beff = b + bprev @ w
            wqkvo[l, pi] = tposed(wf, HT, H).astype(fh)
            if pi == 0:
                params[l, :, 0:6] = _r6(beff)
            elif pi == 1:
                params[l, :, 6:12] = _r6(beff)
            else:
                bvrep[l] = np.broadcast_to(beff, (P, H))
        wao = np.asarray(inputs["Wao"][l], f32)
        wqkvo[l, 3] = tposed(wao, HT, H).astype(fh)
        wi_l = np.asarray(inputs["Wi"][l], f32)
        wi[l] = tposed(wi_l * ln1_g[l][:, None], HT, F).astype(fh)
        bi_eff = np.asarray(inputs["bi"][l], f32) + ln1_b[l] @ wi_l
        wio[l] = tposed(np.asarray(inputs["Wio"][l], f32), FT, H).astype(fh)

        params[l, :, 12:18] = _r6(gprev)
        params[l, :, 18:24] = _r6(ln1_g[l])
        params[l, :, 24:30] = _r6(ln2_g[l])
        params[l, :, 30:36] = _r6(ln2_b[l])
        params[l, :, 48:72] = bi_eff.reshape(FT, P).T
        cr[l, 0, 0:H] = (np.asarray(inputs["bao"][l], f32) + bprev).astype(fh)
        cr[l, 0, H : 2 * H] = (
            np.asarray(inputs["bio"][l], f32) + ln1_b[l]
        ).astype(fh)

    tok = np.ascontiguousarray(np.asarray(inputs["tok_emb"], f32))
    te = np.asarray(inputs["type_emb"], f32)
    posn = np.ascontiguousarray(
        (np.asarray(inputs["pos_emb"], f32)[:S] + te[0])
        .reshape(ST, P, H).transpose(1, 0, 2)
    )
    dte = np.ascontiguousarray(np.broadcast_to(te[1] - te[0], (P, H)))
    return {
        "wqkvo": wqkvo, "wi": wi, "wio": wio, "params": params,
        "cr": cr, "bvrep": bvrep, "tok": tok, "posn": posn, "dte": dte,
    }


def kernel(**inputs):
    if "nc" not in _CACHE:
        _CACHE["nc"] = _build()
    nc = _CACHE["nc"]

    shared = _prep_shared(inputs)
    ids_full = np.asarray(inputs["input_ids"], np.int32)
    seg_full = np.asarray(inputs["segment_ids"], np.int32)
    mask_full = np.asarray(inputs["attention_mask"], np.float32)

    in_maps = []
    for c in range(NCORES):
        m = dict(shared)
        m["ids"] = np.ascontiguousarray(ids_full[c].reshape(ST, P).T)
        m["segf"] = np.ascontiguousarray(
            seg_full[c].astype(np.float32).reshape(ST, P).T
        )
        mrow = (1.0 - mask_full[c, 0, 0]) * -10000.0
        m["maskt"] = np.ascontiguousarray(mrow.reshape(ST, P).T)
        in_maps.append(m)

    res = run_bass_kernel_spmd(nc, in_maps, core_ids=list(range(NCORES)))
    out = np.empty((B, S, H), dtype=np.float32)
    for c in range(NCORES):
        out[c] = res.results[c]["out"].T
    return out


# revision 32
# speedup vs baseline: 1.1793x; 1.1793x over previous
"""BERT-base (12-layer) forward pass on 8 Trainium2 NeuronCores.

Strategy: data-parallel over batch (B=8 -> 1 sequence per core), no
collectives. Host casts weights to bf16 and folds each LayerNorm's gain
into the consuming weight matrices (bias folded into effective biases),
so on-device LayerNorm only produces z = (x - mean) * rstd.

Key scheduling ideas (vs the straightforward version):
- S-split software pipelining: AO/FFN1/FFN2/QK matmul groups process the
  sequence in two halves of 256, so each LayerNorm's serial stats chain
  for half A runs concurrently with matmuls of half B -> the Tensor
  engine never drains (keeps the PE HAM clock at 2.4 GHz).
- LN stats are produced in broadcast form directly: mean/meansq rows are
  computed with M=128 all-ones stationary matmuls, eliminating the
  GpSimd partition-broadcast and [1,S] row ops from the critical chain.
- Residual biases (bao + b_prev, bio + b_ln1) enter PSUM via K=1
  rank-1 matmuls appended to each accumulation group; the residual add
  is then a single scalar_tensor_tensor (g_prev * z_prev + psum).
- Activation-table swaps (exp/gelu/abs_rsqrt) are prefetched with dummy
  1-element activations during matmul phases, off the critical path.
- Paired-head softmax: both heads of a 128-feature block share one
  [P,1024] PSUM tile and a single fused Exp activation.
"""
import sys
import os

if "/opt/trn_rl_repo" not in sys.path:
    sys.path.insert(0, "/opt/trn_rl_repo")

import numpy as np
import ml_dtypes

import concourse.bass as bass
from concourse import bacc
import concourse.tile as tile
from concourse import mybir
from concourse.bass_utils import run_bass_kernel_spmd
from concourse.masks import make_identity

F32 = mybir.dt.float32
F32R = mybir.dt.float32r
BF16 = mybir.dt.bfloat16
FP16 = mybir.dt.float16
INT32 = mybir.dt.int32
AF = mybir.ActivationFunctionType
ALU = mybir.AluOpType

# Model dims (hardcoded per problem spec)
B, S, H, NH, L, F = 8, 512, 768, 12, 12, 3072
V, TV, PP = 21128, 2, 512
DH = H // NH            # 64
P = 128
HT = H // P             # 6
FT = F // P             # 24
ST = S // P             # 4
SH = S // 2             # 256  (sequence half)
EPS = 1e-12
NCORES = 8

NL = int(os.environ.get("BERT_KERNEL_LAYERS", str(L)))

_CACHE = {}


def _build():
    nc = bacc.Bacc("TRN2", target_bir_lowering=False, debug=False)

    # ---- DRAM I/O ----
    WQKVO = nc.dram_tensor("wqkvo", [NL, 4, P, HT, H], FP16, kind="ExternalInput")
    WI = nc.dram_tensor("wi", [NL, P, HT, F], FP16, kind="ExternalInput")
    WIO = nc.dram_tensor("wio", [NL, P, FT, H], FP16, kind="ExternalInput")
    PARAMS = nc.dram_tensor("params", [NL, P, 76], F32, kind="ExternalInput")
    CR = nc.dram_tensor("cr", [NL, 1, 2 * H], FP16, kind="ExternalInput")
    BVREP = nc.dram_tensor("bvrep", [NL, P, H], F32, kind="ExternalInput")
    TOK = nc.dram_tensor("tok", [V, H], F32, kind="ExternalInput")
    POSN = nc.dram_tensor("posn", [P, ST, H], F32, kind="ExternalInput")
    DTE = nc.dram_tensor("dte", [P, H], F32, kind="ExternalInput")
    IDS = nc.dram_tensor("ids", [P, ST], INT32, kind="ExternalInput")
    SEGF = nc.dram_tensor("segf", [P, ST], F32, kind="ExternalInput")
    MASKT = nc.dram_tensor("maskt", [P, ST], F32, kind="ExternalInput")
    OUT = nc.dram_tensor("out", [H, S], F32, kind="ExternalOutput")

    outv = OUT[:].rearrange("(ht p) s -> p ht s", p=P)

    DEBUG_TAPS = int(os.environ.get("BERT_DEBUG_TAPS", "0"))
    TAPS = ["z0", "q", "k", "ctx", "pre1", "z1", "pre2"]
    if DEBUG_TAPS:
        DBG = nc.dram_tensor("dbg", [len(TAPS), P, HT, S], F32,
                             kind="ExternalOutput")

    def hs(h):
        return slice(h * SH, (h + 1) * SH)

    from contextlib import ExitStack

    with tile.TileContext(nc) as tc:
        with ExitStack() as es:
            ec = es.enter_context
            cpool = ec(tc.tile_pool(name="const", bufs=1))
            apool = ec(tc.tile_pool(name="act", bufs=1))     # big per-layer activations
            rpool = ec(tc.tile_pool(name="res", bufs=2))     # z (bf16) ping-pong
            crpool = ec(tc.tile_pool(name="crp", bufs=2))    # bias rows per layer
            scr = ec(tc.tile_pool(name="scr", bufs=6))       # [P,256] scratch
            t1p = ec(tc.tile_pool(name="t1p", bufs=7))       # LN t1 tiles
            rsdpool = ec(tc.tile_pool(name="rsd", bufs=2))   # rstd|mq per half
            rows = ec(tc.tile_pool(name="rows", bufs=2))     # [1,S] rows
            bcp = ec(tc.tile_pool(name="bcast", bufs=4))     # attn bcast temps
            bvpool = ec(tc.tile_pool(name="bv", bufs=1))
            eppool = ec(tc.tile_pool(name="ep", bufs=2))     # exp tiles
            pp = ec(tc.tile_pool(name="pp", bufs=2, space="PSUM"))    # 2 banks
            psc = ec(tc.tile_pool(name="psc", bufs=2, space="PSUM"))  # 4 banks
            pcx = ec(tc.tile_pool(name="pcx", bufs=2, space="PSUM"))  # 2 banks
            # ---- persistent constants ----
            params = cpool.tile([P, NL, 76], F32, tag="params")
            nc.sync.dma_start(params[:], PARAMS[:].rearrange("l p c -> p l c"))
            maskt = cpool.tile([P, ST], F32, tag="maskt")
            nc.sync.dma_start(maskt[:], MASKT[:])
            ones_b = cpool.tile([P, P], BF16, tag="ones")
            nc.any.memset(ones_b[:], 1.0)
            ones_bh = cpool.tile([P, P], BF16, tag="onesbh")
            nc.any.memset(ones_bh[:], 1.0 / H)
            onesrow = cpool.tile([1, S], FP16, tag="onesrow")
            nc.any.memset(onesrow[:], 1.0)
            onesf = cpool.tile([1, P], F32, tag="onesf")
            nc.any.memset(onesf[:], 1.0)
            epsc = cpool.tile([P, 1], F32, tag="epsc")
            nc.any.memset(epsc[:], EPS)
            dscr = cpool.tile([1, 1], F32, tag="dscr")
            nc.any.memset(dscr[:], 0.25)

            # ---- persistent activations ----
            qTb = apool.tile([P, HT, S], FP16, tag="qTb")
            kTb = apool.tile([P, HT, S], FP16, tag="kTb")
            ctxTb = apool.tile([P, HT, S], FP16, tag="ctxTb")
            v_aug = apool.tile([P, ST, NH * (DH + 1)], BF16, tag="v_aug")
            hTb = apool.tile([P, FT, S], FP16, tag="hTb")
            preF = apool.tile([P, HT, S], FP16, tag="preF")

            # ones columns of v_aug (written once; per-layer V writes leave them)
            va_view = v_aug[:].rearrange("p st (h d) -> p st h d", d=DH + 1)
            nc.any.memset(va_view[:, :, :, DH : DH + 1], 1.0)

            # ============ embedding (scoped pool, released after) ============
            xzb = rpool.tile([P, HT, S], FP16, tag="resz")
            with tc.tile_pool(name="embp", bufs=3) as embp:
                posn = embp.tile([P, ST, H], F32, tag="posn", bufs=1)
                nc.sync.dma_start(posn[:], POSN[:])
                dte = embp.tile([P, H], F32, tag="dte", bufs=1)
                nc.sync.dma_start(dte[:], DTE[:])
                ids = embp.tile([P, ST], INT32, tag="ids", bufs=1)
                nc.sync.dma_start(ids[:], IDS[:])
                segf = embp.tile([P, ST], F32, tag="segf", bufs=1)
                nc.sync.dma_start(segf[:], SEGF[:])
                ident = embp.tile([P, P], F32, tag="ident", bufs=1)
                make_identity(nc, ident)

                for st in range(ST):
                    x0 = embp.tile([P, H], F32, tag="x0")
                    nc.gpsimd.indirect_dma_start(
                        out=x0[:],
                        out_offset=None,
                        in_=TOK[:],
                        in_offset=bass.IndirectOffsetOnAxis(
                            ap=ids[:, st : st + 1], axis=0
                        ),
                    )
                    # + (pos + type0) + seg*(type1-type0)
                    tseg = embp.tile([P, H], F32, tag="tseg")
                    nc.scalar.activation(
                        tseg[:], dte[:], AF.Copy, scale=segf[:, st : st + 1]
                    )
                    nc.vector.tensor_add(out=x0[:], in0=x0[:], in1=posn[:, st])
                    nc.gpsimd.tensor_tensor(x0[:], x0[:], tseg[:], ALU.add)
                    # LayerNorm along free dim (features); z only (g,b folded)
                    s1 = embp.tile([P, 1], F32, tag="s1")
                    nc.vector.reduce_sum(s1[:], x0[:], axis=mybir.AxisListType.X)
                    sqs = embp.tile([P, H], F32, tag="sqs")
                    ssq = embp.tile([P, 1], F32, tag="ssq")
                    nc.scalar.activation(sqs[:], x0[:], AF.Square, accum_out=ssq[:])
                    mean = embp.tile([P, 1], F32, tag="mean")
                    nc.any.tensor_scalar_mul(mean[:], s1[:], 1.0 / H)
                    msq = embp.tile([P, 1], F32, tag="msq")
                    nc.any.tensor_scalar_mul(msq[:], ssq[:], 1.0 / H)
                    var = embp.tile([P, 1], F32, tag="var")
                    nc.vector.tensor_tensor(var[:], mean[:], mean[:], ALU.mult)
                    nc.vector.tensor_tensor(var[:], msq[:], var[:], ALU.subtract)
                    rstd = embp.tile([P, 1], F32, tag="rstd")
                    lnv0 = embp.tile([P, 1], F32, tag="lnv0")
                    nc.scalar.activation(lnv0[:], var[:], AF.Ln, bias=epsc[:])
                    nc.scalar.activation(rstd[:], lnv0[:], AF.Exp, scale=-0.5)
                    negmr = embp.tile([P, 1], F32, tag="negmr")
                    nc.vector.tensor_tensor(negmr[:], mean[:], rstd[:], ALU.mult)
                    nc.any.tensor_scalar_mul(negmr[:], negmr[:], -1.0)
                    nc.scalar.activation(
                        x0[:], x0[:], AF.Identity, bias=negmr[:], scale=rstd[:]
                    )
                    # transpose into z layout (bf16)
                    for ht in range(HT):
                        pt = pp.tile([P, S], F32, tag="proj")
                        nc.tensor.transpose(
                            pt[:, :P], x0[:, P * ht : P * (ht + 1)], ident[:]
                        )
                        nc.scalar.copy(xzb[:, ht, P * st : P * (st + 1)], pt[:, :P])

            # ============ transformer layers ============

            def ln_half(pst, h, srcF, l, final):
                """Stats chain + apply for one sequence half.

                pst[:, :SH] = broadcast mean, pst[:, SH:] = broadcast meansq.
                Writes zdst bf16 (or the final DRAM output when `final`).
                Returns the z tile (None when final)."""
                pb = params[:, l, :]
                m2 = scr.tile([P, SH], F32, tag="s")
                nc.scalar.activation(m2[:], pst[:, 0:SH], AF.Square)
                varb = scr.tile([P, SH], F32, tag="s")
                nc.vector.tensor_tensor(varb[:], pst[:, SH:S], m2[:], ALU.subtract)
                rsd = rsdpool.tile([P, S], FP16, tag="rsd")
                nc.scalar.activation(
                    rsd[:, 0:SH], varb[:], AF.Abs_reciprocal_sqrt, bias=epsc[:]
                )
                return rsd

            def ln_sub_mean(pst, h, srcF, t1s):
                # phase A: t1 = x - mean_b (Vector: PSUM read); starts
                # before rstd is ready
                for kt in range(HT):
                    t1 = t1p.tile([P, SH], FP16, tag="t")
                    nc.vector.tensor_tensor(
                        t1[:], srcF[:, kt, hs(h)], pst[:, 0:SH], ALU.subtract
                    )
                    t1s.append(t1)

            def ln_apply(rsd, h, t1s, zdst, l, final):
                pb = params[:, l, :]
                for kt in range(HT):
                    eng = nc.vector
                    if not final:
                        eng.tensor_tensor(
                            zdst[:, kt, hs(h)], t1s[kt][:], rsd[:, 0:SH],
                            ALU.mult,
                        )
                    else:
                        of = scr.tile([P, SH], F32, tag="s")
                        eng.tensor_tensor(of[:], t1s[kt][:], rsd[:, 0:SH],
                                          ALU.mult)
                        nc.scalar.activation(
                            of[:], of[:], AF.Identity,
                            bias=pb[:, 30 + kt : 31 + kt],
                            scale=pb[:, 24 + kt : 25 + kt],
                        )
                        nc.sync.dma_start(outv[:, kt, hs(h)], of[:])

            def tap(name, src, l):
                if not DEBUG_TAPS or l != 0:
                    return
                ti = TAPS.index(name)
                for kt in range(HT):
                    if src.dtype == F32:
                        nc.sync.dma_start(DBG[ti][:, kt], src[:, kt])
                    else:
                        dc = scr.tile([P, S], F32, tag="dbg", bufs=2)
                        nc.scalar.activation(dc[:], src[:, kt], AF.Identity)
                        nc.sync.dma_start(DBG[ti][:, kt], dc[:])

            tap("z0", xzb, 0)

            wqpool = ec(tc.tile_pool(name="wq", bufs=2))
            wfpool = ec(tc.tile_pool(name="wf", bufs=2))
            wopool = ec(tc.tile_pool(name="wo", bufs=1))
            if True:
                for l in range(NL):
                    pb = params[:, l, :]
                    crows = crpool.tile([1, 2 * H], FP16, tag="crows")
                    nc.sync.dma_start(crows[:], CR[l])

                    # ---- Q, K projections (transposed out; consume xzb) ----
                    scope_qk = nc.named_scope(f"qk_{l}"); scope_qk.__enter__()
                    for pi, dst in ((0, qTb), (1, kTb)):
                        w = wqpool.tile([P, HT, H], FP16, tag="wqk")
                        nc.sync.dma_start(w[:], WQKVO[l, pi])
                        for h in range(2):
                            for nt in range(HT):
                                ps = pp.tile([P, S], F32, tag="proj")
                                for kt in range(HT):
                                    nc.tensor.matmul(
                                        ps[:, :SH],
                                        w[:, kt, P * nt : P * (nt + 1)],
                                        xzb[:, kt, hs(h)],
                                        start=(kt == 0), stop=(kt == HT - 1),
                                    )
                                c = 6 * pi + nt
                                nc.vector.tensor_scalar_add(
                                    dst[:, nt, hs(h)], ps[:, :SH],
                                    pb[:, c : c + 1],
                                )

                    scope_qk.__exit__(None, None, None)
                    tap("q", qTb, l)
                    tap("k", kTb, l)
                    scope_v = nc.named_scope(f"v_{l}"); scope_v.__enter__()
                    # ---- V projection (normal layout, augmented buffer) ----
                    nc.scalar.activation(dscr[:], qTb[0:1, 0, 0:1], AF.Exp)
                    wv = wqpool.tile([P, HT, H], FP16, tag="wqk")
                    nc.sync.dma_start(wv[:], WQKVO[l, 2])
                    bvr = bvpool.tile([P, H], F32, tag="bvr")
                    nc.sync.dma_start(bvr[:], BVREP[l])
                    for st in range(ST):
                        for half in range(2):
                            ps = pp.tile([P, S], F32, tag="proj")
                            for kt in range(HT):
                                nc.tensor.matmul(
                                    ps[:, :384],
                                    xzb[:, kt, P * st : P * (st + 1)],
                                    wv[:, kt, 384 * half : 384 * (half + 1)],
                                    start=(kt == 0), stop=(kt == HT - 1),
                                )
                            dst3 = va_view[:, st, 6 * half : 6 * (half + 1), 0:DH]
                            src3 = ps[:, :384].rearrange("p (h d) -> p h d", d=DH)
                            bv3 = bvr[:, 384 * half : 384 * (half + 1)].rearrange(
                                "p (h d) -> p h d", d=DH
                            )
                            nc.vector.tensor_tensor(dst3, src3, bv3, ALU.add)

                    scope_v.__exit__(None, None, None)
                    scope_at = nc.named_scope(f"attn_{l}"); scope_at.__enter__()
                    # ---- attention (head pairs share ht; fused pair Exp) ----
                    for ht in range(HT):
                        exq = eppool.tile([P, ST, 2 * S], BF16, tag="exq")
                        for kti in range(ST):
                            pshh = psc.tile([P, 2 * S], F32, tag="sc")
                            for sub in range(2):
                                base = 64 * sub
                                nc.tensor.matmul(
                                    pshh[:, S * sub : S * (sub + 1)],
                                    kTb[base : base + DH, ht, P * kti : P * (kti + 1)],
                                    qTb[base : base + DH, ht, :],
                                    start=True, stop=True,
                                )
                            nc.scalar.activation(
                                exq[:, kti], pshh[:], AF.Exp,
                                scale=0.125, bias=maskt[:, kti : kti + 1],
                            )
                        for sub in range(2):
                            hh = 2 * ht + sub
                            base = 64 * sub
                            ppc = pcx.tile([DH + 1, S], F32, tag="cx")
                            for kti in range(ST):
                                nc.tensor.matmul(
                                    ppc[:],
                                    v_aug[:, kti, (DH + 1) * hh : (DH + 1) * (hh + 1)],
                                    exq[:, kti, S * sub : S * (sub + 1)],
                                    start=(kti == 0), stop=(kti == ST - 1),
                                )
                            # 1/denominator on the PSUM row directly, then
                            # broadcast via K=1 matmul on the (idle) PE
                            srow0 = rows.tile([1, S], F32, tag="srow0")
                            nc.vector.tensor_copy(srow0[:], ppc[DH : DH + 1, :])
                            rec = rows.tile([1, S], F32, tag="srow0")
                            nc.vector.reciprocal_approx_fast(rec[:], srow0[:])
                            rb = bcp.tile([DH, S], F32, tag="b")
                            nc.gpsimd.partition_broadcast(rb[:], rec[:])
                            nc.vector.tensor_tensor(
                                ctxTb[base : base + DH, ht, :], ppc[:DH, :],
                                rb[:], ALU.mult,
                            )

                    scope_at.__exit__(None, None, None)
                    tap("ctx", ctxTb, l)
                    scope_ao = nc.named_scope(f"ao_{l}"); scope_ao.__enter__()
                    # ---- attention output + residual + LN1 (S-split) ----
                    wao = wqpool.tile([P, HT, H], FP16, tag="wqk")
                    nc.sync.dma_start(wao[:], WQKVO[l, 3])
                    azb = rpool.tile([P, HT, S], FP16, tag="resz")
                    for h in range(2):
                        pst = pcx.tile([P, S], F32, tag="cx")
                        for nt in range(HT):
                            ps = pp.tile([P, S], F32, tag="proj")
                            for kt in range(HT):
                                nc.tensor.matmul(
                                    ps[:, :SH],
                                    wao[:, kt, P * nt : P * (nt + 1)],
                                    ctxTb[:, kt, hs(h)],
                                    start=(kt == 0), stop=False,
                                )
                            # + (bao + b_prev) via rank-1 K=1 matmul
                            nc.tensor.matmul(
                                ps[:, :SH],
                                crows[0:1, P * nt : P * (nt + 1)],
                                onesrow[0:1, :SH],
                                start=False, stop=True,
                            )
                            # pre1 = ps + g_prev * z_prev
                            nc.vector.scalar_tensor_tensor(
                                preF[:, nt, hs(h)], xzb[:, nt, hs(h)],
                                pb[:, 12 + nt : 13 + nt], ps[:, :SH],
                                ALU.mult, ALU.add,
                            )
                            if h == 0 and nt == 0:
                                # dep-pinned abs_rsqrt table prefetch
                                nc.scalar.activation(
                                    dscr[:], preF[0:1, 0, 0:1],
                                    AF.Abs_reciprocal_sqrt,
                                )
                            # stats (broadcast form)
                            # single accumulation group per bank:
                            # sc2 = [x/H | x*x/H] -> [mean_b | msq_b]
                            sc2 = scr.tile([P, S], BF16, tag="s")
                            nc.scalar.activation(
                                sc2[:, 0:SH], preF[:, nt, hs(h)], AF.Copy,
                                scale=1.0 / H,
                            )
                            nc.vector.tensor_tensor(
                                sc2[:, SH:S], sc2[:, 0:SH],
                                preF[:, nt, hs(h)], ALU.mult,
                            )
                            nc.tensor.matmul(
                                pst[:, 0:S], ones_b[:], sc2[:],
                                start=(nt == 0), stop=(nt == HT - 1),
                            )
                        t1s = []
                        ln_sub_mean(pst, h, preF, t1s)
                        rsd = ln_half(pst, h, preF, l, False)
                        ln_apply(rsd, h, t1s, azb, l, False)
                    tap("pre1", preF, l)
                    tap("z1", azb, l)
                    scope_ao.__exit__(None, None, None)

                    scope_f1 = nc.named_scope(f"ffn1_{l}"); scope_f1.__enter__()
                    # ---- FFN intermediate (gelu) ----
                    for quarter in range(4):
                        wih = wfpool.tile([P, HT, F // 4], FP16, tag="wi")
                        nc.sync.dma_start(
                            wih[:],
                            WI[l][:, :, (F // 4) * quarter : (F // 4) * (quarter + 1)],
                        )
                        for h in range(2):
                            for ntl in range(6):
                                nt = 6 * quarter + ntl
                                ps = pp.tile([P, S], F32, tag="proj")
                                for kt in range(HT):
                                    nc.tensor.matmul(
                                        ps[:, :SH],
                                        wih[:, kt, P * ntl : P * (ntl + 1)],
                                        azb[:, kt, hs(h)],
                                        start=(kt == 0), stop=(kt == HT - 1),
                                    )
                                nc.scalar.activation(
                                    hTb[:, nt, hs(h)], ps[:, :SH], AF.Gelu,
                                    bias=pb[:, 48 + nt : 49 + nt],
                                )

                    scope_f1.__exit__(None, None, None)
                    scope_f2 = nc.named_scope(f"ffn2_{l}"); scope_f2.__enter__()
                    # ---- FFN output + residual + LN2 (S-split) ----
                    wio = wopool.tile([P, FT, H], FP16, tag="wio")
                    nc.sync.dma_start(wio[:], WIO[l])
                    final = l == NL - 1
                    if not final:
                        xzb = rpool.tile([P, HT, S], FP16, tag="resz")
                    for h in range(2):
                        pst = pcx.tile([P, S], F32, tag="cx")
                        for nt in range(HT):
                            ps = pp.tile([P, S], F32, tag="proj")
                            for kt in range(FT):
                                nc.tensor.matmul(
                                    ps[:, :SH],
                                    wio[:, kt, P * nt : P * (nt + 1)],
                                    hTb[:, kt, hs(h)],
                                    start=(kt == 0), stop=False,
                                )
                            # + (bio + b_ln1) via rank-1 K=1 matmul
                            nc.tensor.matmul(
                                ps[:, :SH],
                                crows[0:1, H + P * nt : H + P * (nt + 1)],
                                onesrow[0:1, :SH],
                                start=False, stop=True,
                            )
                            # pre2 = ps + g_ln1 * z_attn
                            nc.vector.scalar_tensor_tensor(
                                preF[:, nt, hs(h)], azb[:, nt, hs(h)],
                                pb[:, 18 + nt : 19 + nt], ps[:, :SH],
                                ALU.mult, ALU.add,
                            )
                            if h == 0 and nt == 0:
                                nc.scalar.activation(
                                    dscr[:], preF[0:1, 0, 0:1],
                                    AF.Abs_reciprocal_sqrt,
                                )
                            # single accumulation group per bank:
                            # sc2 = [x/H | x*x/H] -> [mean_b | msq_b]
                            sc2 = scr.tile([P, S], BF16, tag="s")
                            nc.scalar.activation(
                                sc2[:, 0:SH], preF[:, nt, hs(h)], AF.Copy,
                                scale=1.0 / H,
                            )
                            nc.vector.tensor_tensor(
                                sc2[:, SH:S], sc2[:, 0:SH],
                                preF[:, nt, hs(h)], ALU.mult,
                            )
                            nc.tensor.matmul(
                                pst[:, 0:S], ones_b[:], sc2[:],
                                start=(nt == 0), stop=(nt == HT - 1),
                            )
                        t1s = []
                        ln_sub_mean(pst, h, preF, t1s)
                        rsd = ln_half(pst, h, preF, l, final)
                        ln_apply(rsd, h, t1s, None if final else xzb, l, final)
                    tap("pre2", preF, l)
                    scope_f2.__exit__(None, None, None)

    nc.compile()
    return nc


def _r6(v):
    return np.ascontiguousarray(v.reshape(6, P).T)


def _prep_shared(inputs):
    bf = ml_dtypes.bfloat16
    fh = np.float16
    f32 = np.float32
    emb_g = np.asarray(inputs["emb_g"], f32)
    emb_b = np.asarray(inputs["emb_b"], f32)
    ln1_g = np.asarray(inputs["ln1_g"], f32)
    ln1_b = np.asarray(inputs["ln1_b"], f32)
    ln2_g = np.asarray(inputs["ln2_g"], f32)
    ln2_b = np.asarray(inputs["ln2_b"], f32)

    wqkvo = np.empty((NL, 4, P, HT, H), dtype=fh)
    wi = np.empty((NL, P, HT, F), dtype=fh)
    wio = np.empty((NL, P, FT, H), dtype=fh)
    params = np.zeros((NL, P, 76), dtype=f32)
    cr = np.zeros((NL, 1, 2 * H), dtype=fh)
    bvrep = np.empty((NL, P, H), dtype=f32)

    def tposed(w, kt, dim_out):
        # [H, dim_out] -> [P, kt, dim_out]
        return w.reshape(kt, P, dim_out).transpose(1, 0, 2)

    for l in range(NL):
        gprev = emb_g if l == 0 else ln2_g[l - 1]
        bprev = emb_b if l == 0 else ln2_b[l - 1]
        for pi, name, bname in ((0, "Wq", "bq"), (1, "Wk", "bk"), (2, "Wv", "bv")):
            w = np.asarray(inputs[name][l], f32)
            b = np.asarray(inputs[bname][l], f32)
            wf = w * gprev[:, None]
            beff = b + bprev @ w
            wqkvo[l, pi] = tposed(wf, HT, H).astype(fh)
            if pi == 0:
                params[l, :, 0:6] = _r6(beff)
            elif pi == 1:
                params[l, :, 6:12] = _r6(beff)
            else:
                bvrep[l] = np.broadcast_to(beff, (P, H))
        wao = np.asarray(inputs["Wao"][l], f32)
        wqkvo[l, 3] = tposed(wao, HT, H).astype(fh)
        wi_l = np.asarray(inputs["Wi"][l], f32)
        wi[l] = tposed(wi_l * ln1_g[l][:, None], HT, F).astype(fh)
        bi_eff = np.asarray(inputs["bi"][l], f32) + ln1_b[l] @ wi_l
        wio[l] = tposed(np.asarray(inputs["Wio"][l], f32), FT, H).astype(fh)

        params[l, :, 12:18] = _r6(gprev)
        params[l, :, 18:24] = _r6(ln1_g[l])
        params[l, :, 24:30] = _r6(ln2_g[l])
        params[l, :, 30:36] = _r6(ln2_b[l])
        params[l, :, 48:72] = bi_eff.reshape(FT, P).T
        cr[l, 0, 0:H] = (np.asarray(inputs["bao"][l], f32) + bprev).astype(fh)
        cr[l, 0, H : 2 * H] = (
            np.asarray(inputs["bio"][l], f32) + ln1_b[l]
        ).astype(fh)

    tok = np.ascontiguousarray(np.asarray(inputs["tok_emb"], f32))
    te = np.asarray(inputs["type_emb"], f32)
    posn = np.ascontiguousarray(
        (np.asarray(inputs["pos_emb"], f32)[:S] + te[0])
        .reshape(ST, P, H).transpose(1, 0, 2)
    )
    dte = np.ascontiguousarray(np.broadcast_to(te[1] - te[0], (P, H)))
    return {
        "wqkvo": wqkvo, "wi": wi, "wio": wio, "params": params,
        "cr": cr, "bvrep": bvrep, "tok": tok, "posn": posn, "dte": dte,
    }


def kernel(**inputs):
    if "nc" not in _CACHE:
        _CACHE["nc"] = _build()
    nc = _CACHE["nc"]

    shared = _prep_shared(inputs)
    ids_full = np.asarray(inputs["input_ids"], np.int32)
    seg_full = np.asarray(inputs["segment_ids"], np.int32)
    mask_full = np.asarray(inputs["attention_mask"], np.float32)

    in_maps = []
    for c in range(NCORES):
        m = dict(shared)
        m["ids"] = np.ascontiguousarray(ids_full[c].reshape(ST, P).T)
        m["segf"] = np.ascontiguousarray(
            seg_full[c].astype(np.float32).reshape(ST, P).T
        )
        mrow = (1.0 - mask_full[c, 0, 0]) * -10000.0
        m["maskt"] = np.ascontiguousarray(mrow.reshape(ST, P).T)
        in_maps.append(m)

    res = run_bass_kernel_spmd(nc, in_maps, core_ids=list(range(NCORES)))
    out = np.empty((B, S, H), dtype=np.float32)
    for c in range(NCORES):
        out[c] = res.results[c]["out"].T
    return out


# revision 33
# speedup vs baseline: 1.1840x; 1.0040x over previous
"""BERT-base (12-layer) forward pass on 8 Trainium2 NeuronCores.

Strategy: data-parallel over batch (B=8 -> 1 sequence per core), no
collectives. Host casts weights to bf16 and folds each LayerNorm's gain
into the consuming weight matrices (bias folded into effective biases),
so on-device LayerNorm only produces z = (x - mean) * rstd.

Key scheduling ideas (vs the straightforward version):
- S-split software pipelining: AO/FFN1/FFN2/QK matmul groups process the
  sequence in two halves of 256, so each LayerNorm's serial stats chain
  for half A runs concurrently with matmuls of half B -> the Tensor
  engine never drains (keeps the PE HAM clock at 2.4 GHz).
- LN stats are produced in broadcast form directly: mean/meansq rows are
  computed with M=128 all-ones stationary matmuls, eliminating the
  GpSimd partition-broadcast and [1,S] row ops from the critical chain.
- Residual biases (bao + b_prev, bio + b_ln1) enter PSUM via K=1
  rank-1 matmuls appended to each accumulation group; the residual add
  is then a single scalar_tensor_tensor (g_prev * z_prev + psum).
- Activation-table swaps (exp/gelu/abs_rsqrt) are prefetched with dummy
  1-element activations during matmul phases, off the critical path.
- Paired-head softmax: both heads of a 128-feature block share one
  [P,1024] PSUM tile and a single fused Exp activation.
"""
import sys
import os

if "/opt/trn_rl_repo" not in sys.path:
    sys.path.insert(0, "/opt/trn_rl_repo")

import numpy as np
import ml_dtypes

import concourse.bass as bass
from concourse import bacc
import concourse.tile as tile
from concourse import mybir
from concourse.bass_utils import run_bass_kernel_spmd
from concourse.masks import make_identity

F32 = mybir.dt.float32
F32R = mybir.dt.float32r
BF16 = mybir.dt.bfloat16
FP16 = mybir.dt.float16
INT32 = mybir.dt.int32
AF = mybir.ActivationFunctionType
ALU = mybir.AluOpType

# Model dims (hardcoded per problem spec)
B, S, H, NH, L, F = 8, 512, 768, 12, 12, 3072
V, TV, PP = 21128, 2, 512
DH = H // NH            # 64
P = 128
HT = H // P             # 6
FT = F // P             # 24
ST = S // P             # 4
SH = S // 2             # 256  (sequence half)
EPS = 1e-12
NCORES = 8

NL = int(os.environ.get("BERT_KERNEL_LAYERS", str(L)))

_CACHE = {}


def _build():
    nc = bacc.Bacc("TRN2", target_bir_lowering=False, debug=False)

    # ---- DRAM I/O ----
    WQKVO = nc.dram_tensor("wqkvo", [NL, 4, P, HT, H], FP16, kind="ExternalInput")
    WI = nc.dram_tensor("wi", [NL, P, HT, F], FP16, kind="ExternalInput")
    WIO = nc.dram_tensor("wio", [NL, P, FT, H], FP16, kind="ExternalInput")
    PARAMS = nc.dram_tensor("params", [NL, P, 76], F32, kind="ExternalInput")
    CR = nc.dram_tensor("cr", [NL, 1, 2 * H], FP16, kind="ExternalInput")
    BVREP = nc.dram_tensor("bvrep", [NL, P, H], F32, kind="ExternalInput")
    TOK = nc.dram_tensor("tok", [V, H], F32, kind="ExternalInput")
    POSN = nc.dram_tensor("posn", [P, ST, H], F32, kind="ExternalInput")
    DTE = nc.dram_tensor("dte", [P, H], F32, kind="ExternalInput")
    IDS = nc.dram_tensor("ids", [P, ST], INT32, kind="ExternalInput")
    SEGF = nc.dram_tensor("segf", [P, ST], F32, kind="ExternalInput")
    MASKT = nc.dram_tensor("maskt", [P, ST], F32, kind="ExternalInput")
    OUT = nc.dram_tensor("out", [H, S], F32, kind="ExternalOutput")

    outv = OUT[:].rearrange("(ht p) s -> p ht s", p=P)

    DEBUG_TAPS = int(os.environ.get("BERT_DEBUG_TAPS", "0"))
    TAPS = ["z0", "q", "k", "ctx", "pre1", "z1", "pre2"]
    if DEBUG_TAPS:
        DBG = nc.dram_tensor("dbg", [len(TAPS), P, HT, S], F32,
                             kind="ExternalOutput")

    def hs(h):
        return slice(h * SH, (h + 1) * SH)

    from contextlib import ExitStack

    with tile.TileContext(nc) as tc:
        with ExitStack() as es:
            ec = es.enter_context
            cpool = ec(tc.tile_pool(name="const", bufs=1))
            apool = ec(tc.tile_pool(name="act", bufs=1))     # big per-layer activations
            rpool = ec(tc.tile_pool(name="res", bufs=2))     # z (bf16) ping-pong
            crpool = ec(tc.tile_pool(name="crp", bufs=2))    # bias rows per layer
            scr = ec(tc.tile_pool(name="scr", bufs=6))       # [P,256] scratch
            t1p = ec(tc.tile_pool(name="t1p", bufs=7))       # LN t1 tiles
            rsdpool = ec(tc.tile_pool(name="rsd", bufs=2))   # rstd|mq per half
            rows = ec(tc.tile_pool(name="rows", bufs=2))     # [1,S] rows
            bcp = ec(tc.tile_pool(name="bcast", bufs=4))     # attn bcast temps
            bvpool = ec(tc.tile_pool(name="bv", bufs=1))
            eppool = ec(tc.tile_pool(name="ep", bufs=2))     # exp tiles
            pp = ec(tc.tile_pool(name="pp", bufs=2, space="PSUM"))    # 2 banks
            psc = ec(tc.tile_pool(name="psc", bufs=2, space="PSUM"))  # 4 banks
            pcx = ec(tc.tile_pool(name="pcx", bufs=2, space="PSUM"))  # 2 banks
            # ---- persistent constants ----
            params = cpool.tile([P, NL, 76], F32, tag="params")
            nc.sync.dma_start(params[:], PARAMS[:].rearrange("l p c -> p l c"))
            maskt = cpool.tile([P, ST], F32, tag="maskt")
            nc.sync.dma_start(maskt[:], MASKT[:])
            ones_b = cpool.tile([P, P], BF16, tag="ones")
            nc.any.memset(ones_b[:], 1.0)
            ones_bh = cpool.tile([P, P], BF16, tag="onesbh")
            nc.any.memset(ones_bh[:], 1.0 / H)
            onesrow = cpool.tile([1, S], FP16, tag="onesrow")
            nc.any.memset(onesrow[:], 1.0)
            onesf = cpool.tile([1, P], F32, tag="onesf")
            nc.any.memset(onesf[:], 1.0)
            epsc = cpool.tile([P, 1], F32, tag="epsc")
            nc.any.memset(epsc[:], EPS)
            dscr = cpool.tile([1, 1], F32, tag="dscr")
            nc.any.memset(dscr[:], 0.25)

            # ---- persistent activations ----
            qTb = apool.tile([P, HT, S], FP16, tag="qTb")
            kTb = apool.tile([P, HT, S], FP16, tag="kTb")
            ctxTb = apool.tile([P, HT, S], FP16, tag="ctxTb")
            v_aug = apool.tile([P, ST, NH * (DH + 1)], BF16, tag="v_aug")
            hTb = apool.tile([P, FT, S], FP16, tag="hTb")
            preF = apool.tile([P, HT, S], FP16, tag="preF")

            # ones columns of v_aug (written once; per-layer V writes leave them)
            va_view = v_aug[:].rearrange("p st (h d) -> p st h d", d=DH + 1)
            nc.any.memset(va_view[:, :, :, DH : DH + 1], 1.0)

            # ============ embedding (scoped pool, released after) ============
            xzb = rpool.tile([P, HT, S], FP16, tag="resz")
            with tc.tile_pool(name="embp", bufs=3) as embp:
                posn = embp.tile([P, ST, H], F32, tag="posn", bufs=1)
                nc.sync.dma_start(posn[:], POSN[:])
                dte = embp.tile([P, H], F32, tag="dte", bufs=1)
                nc.sync.dma_start(dte[:], DTE[:])
                ids = embp.tile([P, ST], INT32, tag="ids", bufs=1)
                nc.sync.dma_start(ids[:], IDS[:])
                segf = embp.tile([P, ST], F32, tag="segf", bufs=1)
                nc.sync.dma_start(segf[:], SEGF[:])
                ident = embp.tile([P, P], F32, tag="ident", bufs=1)
                make_identity(nc, ident)

                for st in range(ST):
                    x0 = embp.tile([P, H], F32, tag="x0")
                    nc.gpsimd.indirect_dma_start(
                        out=x0[:],
                        out_offset=None,
                        in_=TOK[:],
                        in_offset=bass.IndirectOffsetOnAxis(
                            ap=ids[:, st : st + 1], axis=0
                        ),
                    )
                    # + (pos + type0) + seg*(type1-type0)
                    tseg = embp.tile([P, H], F32, tag="tseg")
                    nc.scalar.activation(
                        tseg[:], dte[:], AF.Copy, scale=segf[:, st : st + 1]
                    )
                    nc.vector.tensor_add(out=x0[:], in0=x0[:], in1=posn[:, st])
                    nc.gpsimd.tensor_tensor(x0[:], x0[:], tseg[:], ALU.add)
                    # LayerNorm along free dim (features); z only (g,b folded)
                    s1 = embp.tile([P, 1], F32, tag="s1")
                    nc.vector.reduce_sum(s1[:], x0[:], axis=mybir.AxisListType.X)
                    sqs = embp.tile([P, H], F32, tag="sqs")
                    ssq = embp.tile([P, 1], F32, tag="ssq")
                    nc.scalar.activation(sqs[:], x0[:], AF.Square, accum_out=ssq[:])
                    mean = embp.tile([P, 1], F32, tag="mean")
                    nc.any.tensor_scalar_mul(mean[:], s1[:], 1.0 / H)
                    msq = embp.tile([P, 1], F32, tag="msq")
                    nc.any.tensor_scalar_mul(msq[:], ssq[:], 1.0 / H)
                    var = embp.tile([P, 1], F32, tag="var")
                    nc.vector.tensor_tensor(var[:], mean[:], mean[:], ALU.mult)
                    nc.vector.tensor_tensor(var[:], msq[:], var[:], ALU.subtract)
                    rstd = embp.tile([P, 1], F32, tag="rstd")
                    lnv0 = embp.tile([P, 1], F32, tag="lnv0")
                    nc.scalar.activation(lnv0[:], var[:], AF.Ln, bias=epsc[:])
                    nc.scalar.activation(rstd[:], lnv0[:], AF.Exp, scale=-0.5)
                    negmr = embp.tile([P, 1], F32, tag="negmr")
                    nc.vector.tensor_tensor(negmr[:], mean[:], rstd[:], ALU.mult)
                    nc.any.tensor_scalar_mul(negmr[:], negmr[:], -1.0)
                    nc.scalar.activation(
                        x0[:], x0[:], AF.Identity, bias=negmr[:], scale=rstd[:]
                    )
                    # transpose into z layout (bf16)
                    for ht in range(HT):
                        pt = pp.tile([P, S], F32, tag="proj")
                        nc.tensor.transpose(
                            pt[:, :P], x0[:, P * ht : P * (ht + 1)], ident[:]
                        )
                        nc.scalar.copy(xzb[:, ht, P * st : P * (st + 1)], pt[:, :P])

            # ============ transformer layers ============

            def ln_half(pst, h, srcF, l, final):
                """Stats chain + apply for one sequence half.

                pst[:, :SH] = broadcast mean, pst[:, SH:] = broadcast meansq.
                Writes zdst bf16 (or the final DRAM output when `final`).
                Returns the z tile (None when final)."""
                pb = params[:, l, :]
                m2 = scr.tile([P, SH], F32, tag="s")
                nc.scalar.activation(m2[:], pst[:, 0:SH], AF.Square)
                varb = scr.tile([P, SH], F32, tag="s")
                nc.vector.tensor_tensor(varb[:], pst[:, SH:S], m2[:], ALU.subtract)
                rsd = rsdpool.tile([P, S], FP16, tag="rsd")
                nc.scalar.activation(
                    rsd[:, 0:SH], varb[:], AF.Abs_reciprocal_sqrt, bias=epsc[:]
                )
                return rsd

            def ln_sub_mean(pst, h, srcF, t1s):
                # phase A: t1 = x - mean_b (Vector: PSUM read); starts
                # before rstd is ready
                for kt in range(HT):
                    t1 = t1p.tile([P, SH], FP16, tag="t")
                    nc.vector.tensor_tensor(
                        t1[:], srcF[:, kt, hs(h)], pst[:, 0:SH], ALU.subtract
                    )
                    t1s.append(t1)

            def ln_apply(rsd, h, t1s, zdst, l, final):
                pb = params[:, l, :]
                for kt in range(HT):
                    eng = nc.vector
                    if not final:
                        eng.tensor_tensor(
                            zdst[:, kt, hs(h)], t1s[kt][:], rsd[:, 0:SH],
                            ALU.mult,
                        )
                    else:
                        of = scr.tile([P, SH], F32, tag="s")
                        eng.tensor_tensor(of[:], t1s[kt][:], rsd[:, 0:SH],
                                          ALU.mult)
                        nc.scalar.activation(
                            of[:], of[:], AF.Identity,
                            bias=pb[:, 30 + kt : 31 + kt],
                            scale=pb[:, 24 + kt : 25 + kt],
                        )
                        nc.sync.dma_start(outv[:, kt, hs(h)], of[:])

            def tap(name, src, l):
                if not DEBUG_TAPS or l != 0:
                    return
                ti = TAPS.index(name)
                for kt in range(HT):
                    if src.dtype == F32:
                        nc.sync.dma_start(DBG[ti][:, kt], src[:, kt])
                    else:
                        dc = scr.tile([P, S], F32, tag="dbg", bufs=2)
                        nc.scalar.activation(dc[:], src[:, kt], AF.Identity)
                        nc.sync.dma_start(DBG[ti][:, kt], dc[:])

            tap("z0", xzb, 0)

            wqpool = ec(tc.tile_pool(name="wq", bufs=3))
            wfpool = ec(tc.tile_pool(name="wf", bufs=2))
            wopool = ec(tc.tile_pool(name="wo", bufs=1))
            if True:
                for l in range(NL):
                    pb = params[:, l, :]
                    crows = crpool.tile([1, 2 * H], FP16, tag="crows")
                    nc.sync.dma_start(crows[:], CR[l])

                    # ---- Q, K projections (transposed out; consume xzb) ----
                    scope_qk = nc.named_scope(f"qk_{l}"); scope_qk.__enter__()
                    for pi, dst in ((0, qTb), (1, kTb)):
                        w = wqpool.tile([P, HT, H], FP16, tag="wqk")
                        nc.sync.dma_start(w[:], WQKVO[l, pi])
                        for h in range(2):
                            for nt in range(HT):
                                ps = pp.tile([P, S], F32, tag="proj")
                                for kt in range(HT):
                                    nc.tensor.matmul(
                                        ps[:, :SH],
                                        w[:, kt, P * nt : P * (nt + 1)],
                                        xzb[:, kt, hs(h)],
                                        start=(kt == 0), stop=(kt == HT - 1),
                                    )
                                c = 6 * pi + nt
                                nc.vector.tensor_scalar_add(
                                    dst[:, nt, hs(h)], ps[:, :SH],
                                    pb[:, c : c + 1],
                                )

                    scope_qk.__exit__(None, None, None)
                    tap("q", qTb, l)
                    tap("k", kTb, l)
                    scope_v = nc.named_scope(f"v_{l}"); scope_v.__enter__()
                    # ---- V projection (normal layout, augmented buffer) ----
                    nc.scalar.activation(dscr[:], qTb[0:1, 0, 0:1], AF.Exp)
                    wv = wqpool.tile([P, HT, H], FP16, tag="wqk")
                    nc.sync.dma_start(wv[:], WQKVO[l, 2])
                    bvr = bvpool.tile([P, H], F32, tag="bvr")
                    nc.sync.dma_start(bvr[:], BVREP[l])
                    for st in range(ST):
                        for half in range(2):
                            ps = pp.tile([P, S], F32, tag="proj")
                            for kt in range(HT):
                                nc.tensor.matmul(
                                    ps[:, :384],
                                    xzb[:, kt, P * st : P * (st + 1)],
                                    wv[:, kt, 384 * half : 384 * (half + 1)],
                                    start=(kt == 0), stop=(kt == HT - 1),
                                )
                            dst3 = va_view[:, st, 6 * half : 6 * (half + 1), 0:DH]
                            src3 = ps[:, :384].rearrange("p (h d) -> p h d", d=DH)
                            bv3 = bvr[:, 384 * half : 384 * (half + 1)].rearrange(
                                "p (h d) -> p h d", d=DH
                            )
                            nc.vector.tensor_tensor(dst3, src3, bv3, ALU.add)

                    scope_v.__exit__(None, None, None)
                    scope_at = nc.named_scope(f"attn_{l}"); scope_at.__enter__()
                    # ---- attention (head pairs share ht; fused pair Exp) ----
                    for ht in range(HT):
                        exq = eppool.tile([P, ST, 2 * S], BF16, tag="exq")
                        for kti in range(ST):
                            pshh = psc.tile([P, 2 * S], F32, tag="sc")
                            for sub in range(2):
                                base = 64 * sub
                                nc.tensor.matmul(
                                    pshh[:, S * sub : S * (sub + 1)],
                                    kTb[base : base + DH, ht, P * kti : P * (kti + 1)],
                                    qTb[base : base + DH, ht, :],
                                    start=True, stop=True,
                                )
                            nc.scalar.activation(
                                exq[:, kti], pshh[:], AF.Exp,
                                scale=0.125, bias=maskt[:, kti : kti + 1],
                            )
                        for sub in range(2):
                            hh = 2 * ht + sub
                            base = 64 * sub
                            ppc = pcx.tile([DH + 1, S], F32, tag="cx")
                            for kti in range(ST):
                                nc.tensor.matmul(
                                    ppc[:],
                                    v_aug[:, kti, (DH + 1) * hh : (DH + 1) * (hh + 1)],
                                    exq[:, kti, S * sub : S * (sub + 1)],
                                    start=(kti == 0), stop=(kti == ST - 1),
                                )
                            # 1/denominator on the PSUM row directly, then
                            # broadcast via K=1 matmul on the (idle) PE
                            srow0 = rows.tile([1, S], F32, tag="srow0")
                            nc.vector.tensor_copy(srow0[:], ppc[DH : DH + 1, :])
                            rec = rows.tile([1, S], F32, tag="srow0")
                            nc.vector.reciprocal_approx_fast(rec[:], srow0[:])
                            rb = bcp.tile([DH, S], F32, tag="b")
                            nc.gpsimd.partition_broadcast(rb[:], rec[:])
                            nc.vector.tensor_tensor(
                                ctxTb[base : base + DH, ht, :], ppc[:DH, :],
                                rb[:], ALU.mult,
                            )

                    scope_at.__exit__(None, None, None)
                    tap("ctx", ctxTb, l)
                    scope_ao = nc.named_scope(f"ao_{l}"); scope_ao.__enter__()
                    # ---- attention output + residual + LN1 (S-split) ----
                    wao = wqpool.tile([P, HT, H], FP16, tag="wqk")
                    nc.sync.dma_start(wao[:], WQKVO[l, 3])
                    azb = rpool.tile([P, HT, S], FP16, tag="resz")
                    for h in range(2):
                        pst = pcx.tile([P, S], F32, tag="cx")
                        for nt in range(HT):
                            ps = pp.tile([P, S], F32, tag="proj")
                            for kt in range(HT):
                                nc.tensor.matmul(
                                    ps[:, :SH],
                                    wao[:, kt, P * nt : P * (nt + 1)],
                                    ctxTb[:, kt, hs(h)],
                                    start=(kt == 0), stop=False,
                                )
                            # + (bao + b_prev) via rank-1 K=1 matmul
                            nc.tensor.matmul(
                                ps[:, :SH],
                                crows[0:1, P * nt : P * (nt + 1)],
                                onesrow[0:1, :SH],
                                start=False, stop=True,
                            )
                            # pre1 = ps + g_prev * z_prev
                            nc.vector.scalar_tensor_tensor(
                                preF[:, nt, hs(h)], xzb[:, nt, hs(h)],
                                pb[:, 12 + nt : 13 + nt], ps[:, :SH],
                                ALU.mult, ALU.add,
                            )
                            if h == 0 and nt == 0:
                                # dep-pinned abs_rsqrt table prefetch
                                nc.scalar.activation(
                                    dscr[:], preF[0:1, 0, 0:1],
                                    AF.Abs_reciprocal_sqrt,
                                )
                            # stats (broadcast form)
                            # single accumulation group per bank:
                            # sc2 = [x/H | x*x/H] -> [mean_b | msq_b]
                            sc2 = scr.tile([P, S], BF16, tag="s")
                            nc.scalar.activation(
                                sc2[:, 0:SH], preF[:, nt, hs(h)], AF.Copy,
                                scale=1.0 / H,
                            )
                            nc.vector.tensor_tensor(
                                sc2[:, SH:S], sc2[:, 0:SH],
                                preF[:, nt, hs(h)], ALU.mult,
                            )
                            nc.tensor.matmul(
                                pst[:, 0:S], ones_b[:], sc2[:],
                                start=(nt == 0), stop=(nt == HT - 1),
                            )
                        t1s = []
                        ln_sub_mean(pst, h, preF, t1s)
                        rsd = ln_half(pst, h, preF, l, False)
                        ln_apply(rsd, h, t1s, azb, l, False)
                    tap("pre1", preF, l)
                    tap("z1", azb, l)
                    scope_ao.__exit__(None, None, None)

                    scope_f1 = nc.named_scope(f"ffn1_{l}"); scope_f1.__enter__()
                    # ---- FFN intermediate (gelu) ----
                    for quarter in range(4):
                        wih = wfpool.tile([P, HT, F // 4], FP16, tag="wi")
                        nc.sync.dma_start(
                            wih[:],
                            WI[l][:, :, (F // 4) * quarter : (F // 4) * (quarter + 1)],
                        )
                        for h in range(2):
                            for ntl in range(6):
                                nt = 6 * quarter + ntl
                                ps = pp.tile([P, S], F32, tag="proj")
                                for kt in range(HT):
                                    nc.tensor.matmul(
                                        ps[:, :SH],
                                        wih[:, kt, P * ntl : P * (ntl + 1)],
                                        azb[:, kt, hs(h)],
                                        start=(kt == 0), stop=(kt == HT - 1),
                                    )
                                nc.scalar.activation(
                                    hTb[:, nt, hs(h)], ps[:, :SH], AF.Gelu,
                                    bias=pb[:, 48 + nt : 49 + nt],
                                )

                    scope_f1.__exit__(None, None, None)
                    scope_f2 = nc.named_scope(f"ffn2_{l}"); scope_f2.__enter__()
                    # ---- FFN output + residual + LN2 (S-split) ----
                    wio = wopool.tile([P, FT, H], FP16, tag="wio")
                    nc.sync.dma_start(wio[:], WIO[l])
                    final = l == NL - 1
                    if not final:
                        xzb = rpool.tile([P, HT, S], FP16, tag="resz")
                    for h in range(2):
                        pst = pcx.tile([P, S], F32, tag="cx")
                        for nt in range(HT):
                            ps = pp.tile([P, S], F32, tag="proj")
                            for kt in range(FT):
                                nc.tensor.matmul(
                                    ps[:, :SH],
                                    wio[:, kt, P * nt : P * (nt + 1)],
                                    hTb[:, kt, hs(h)],
                                    start=(kt == 0), stop=False,
                                )
                            # + (bio + b_ln1) via rank-1 K=1 matmul
                            nc.tensor.matmul(
                                ps[:, :SH],
                                crows[0:1, H + P * nt : H + P * (nt + 1)],
                                onesrow[0:1, :SH],
                                start=False, stop=True,
                            )
                            # pre2 = ps + g_ln1 * z_attn
                            nc.vector.scalar_tensor_tensor(
                                preF[:, nt, hs(h)], azb[:, nt, hs(h)],
                                pb[:, 18 + nt : 19 + nt], ps[:, :SH],
                                ALU.mult, ALU.add,
                            )
                            if h == 0 and nt == 0:
                                nc.scalar.activation(
                                    dscr[:], preF[0:1, 0, 0:1],
                                    AF.Abs_reciprocal_sqrt,
                                )
                            # single accumulation group per bank:
                            # sc2 = [x/H | x*x/H] -> [mean_b | msq_b]
                            sc2 = scr.tile([P, S], BF16, tag="s")
                            nc.scalar.activation(
                                sc2[:, 0:SH], preF[:, nt, hs(h)], AF.Copy,
                                scale=1.0 / H,
                            )
                            nc.vector.tensor_tensor(
                                sc2[:, SH:S], sc2[:, 0:SH],
                                preF[:, nt, hs(h)], ALU.mult,
                            )
                            nc.tensor.matmul(
                                pst[:, 0:S], ones_b[:], sc2[:],
                                start=(nt == 0), stop=(nt == HT - 1),
                            )
                        t1s = []
                        ln_sub_mean(pst, h, preF, t1s)
                        rsd = ln_half(pst, h, preF, l, final)
                        ln_apply(rsd, h, t1s, None if final else xzb, l, final)
                    tap("pre2", preF, l)
                    scope_f2.__exit__(None, None, None)

    nc.compile()
    return nc


def _r6(v):
    return np.ascontiguousarray(v.reshape(6, P).T)


def _prep_shared(inputs):
    bf = ml_dtypes.bfloat16
    fh = np.float16
    f32 = np.float32
    emb_g = np.asarray(inputs["emb_g"], f32)
    emb_b = np.asarray(inputs["emb_b"], f32)
    ln1_g = np.asarray(inputs["ln1_g"], f32)
    ln1_b = np.asarray(inputs["ln1_b"], f32)
    ln2_g = np.asarray(inputs["ln2_g"], f32)
    ln2_b = np.asarray(inputs["ln2_b"], f32)

    wqkvo = np.empty((NL, 4, P, HT, H), dtype=fh)
    wi = np.empty((NL, P, HT, F), dtype=fh)
    wio = np.empty((NL, P, FT, H), dtype=fh)
    params = np.zeros((NL, P, 76), dtype=f32)
    cr = np.zeros((NL, 1, 2 * H), dtype=fh)
    bvrep = np.empty((NL, P, H), dtype=f32)

    def tposed(w, kt, dim_out):
        # [H, dim_out] -> [P, kt, dim_out]
        return w.reshape(kt, P, dim_out).transpose(1, 0, 2)

    for l in range(NL):
        gprev = emb_g if l == 0 else ln2_g[l - 1]
        bprev = emb_b if l == 0 else ln2_b[l - 1]
        for pi, name, bname in ((0, "Wq", "bq"), (1, "Wk", "bk"), (2, "Wv", "bv")):
            w = np.asarray(inputs[name][l], f32)
            b = np.asarray(inputs[bname][l], f32)
            wf = w * gprev[:, None]
            beff = b + bprev @ w
            wqkvo[l, pi] = tposed(wf, HT, H).astype(fh)
            if pi == 0:
                params[l, :, 0:6] = _r6(beff)
            elif pi == 1:
                params[l, :, 6:12] = _r6(beff)
            else:
                bvrep[l] = np.broadcast_to(beff, (P, H))
        wao = np.asarray(inputs["Wao"][l], f32)
        wqkvo[l, 3] = tposed(wao, HT, H).astype(fh)
        wi_l = np.asarray(inputs["Wi"][l], f32)
        wi[l] = tposed(wi_l * ln1_g[l][:, None], HT, F).astype(fh)
        bi_eff = np.asarray(inputs["bi"][l], f32) + ln1_b[l] @ wi_l
        wio[l] = tposed(np.asarray(inputs["Wio"][l], f32), FT, H).astype(fh)

        params[l, :, 12:18] = _r6(gprev)
        params[l, :, 18:24] = _r6(ln1_g[l])
        params[l, :, 24:30] = _r6(ln2_g[l])
        params[l, :, 30:36] = _r6(ln2_b[l])
        params[l, :, 48:72] = bi_eff.reshape(FT, P).T
        cr[l, 0, 0:H] = (np.asarray(inputs["bao"][l], f32) + bprev).astype(fh)
        cr[l, 0, H : 2 * H] = (
            np.asarray(inputs["bio"][l], f32) + ln1_b[l]
        ).astype(fh)

    tok = np.ascontiguousarray(np.asarray(inputs["tok_emb"], f32))
    te = np.asarray(inputs["type_emb"], f32)
    posn = np.ascontiguousarray(
        (np.asarray(inputs["pos_emb"], f32)[:S] + te[0])
        .reshape(ST, P, H).transpose(1, 0, 2)
    )
    dte = np.ascontiguousarray(np.broadcast_to(te[1] - te[0], (P, H)))
    return {
        "wqkvo": wqkvo, "wi": wi, "wio": wio, "params": params,
        "cr": cr, "bvrep": bvrep, "tok": tok, "posn": posn, "dte": dte,
    }


def kernel(**inputs):
    if "nc" not in _CACHE:
        _CACHE["nc"] = _build()
    nc = _CACHE["nc"]

    shared = _prep_shared(inputs)
    ids_full = np.asarray(inputs["input_ids"], np.int32)
    seg_full = np.asarray(inputs["segment_ids"], np.int32)
    mask_full = np.asarray(inputs["attention_mask"], np.float32)

    in_maps = []
    for c in range(NCORES):
        m = dict(shared)
        m["ids"] = np.ascontiguousarray(ids_full[c].reshape(ST, P).T)
        m["segf"] = np.ascontiguousarray(
            seg_full[c].astype(np.float32).reshape(ST, P).T
        )
        mrow = (1.0 - mask_full[c, 0, 0]) * -10000.0
        m["maskt"] = np.ascontiguousarray(mrow.reshape(ST, P).T)
        in_maps.append(m)

    res = run_bass_kernel_spmd(nc, in_maps, core_ids=list(range(NCORES)))
    out = np.empty((B, S, H), dtype=np.float32)
    for c in range(NCORES):
        out[c] = res.results[c]["out"].T
    return out


# revision 34
# speedup vs baseline: 1.2304x; 1.0392x over previous
"""BERT-base (12-layer) forward pass on 8 Trainium2 NeuronCores.

Strategy: data-parallel over batch (B=8 -> 1 sequence per core), no
collectives. Host casts weights to bf16 and folds each LayerNorm's gain
into the consuming weight matrices (bias folded into effective biases),
so on-device LayerNorm only produces z = (x - mean) * rstd.

Key scheduling ideas (vs the straightforward version):
- S-split software pipelining: AO/FFN1/FFN2/QK matmul groups process the
  sequence in two halves of 256, so each LayerNorm's serial stats chain
  for half A runs concurrently with matmuls of half B -> the Tensor
  engine never drains (keeps the PE HAM clock at 2.4 GHz).
- LN stats are produced in broadcast form directly: mean/meansq rows are
  computed with M=128 all-ones stationary matmuls, eliminating the
  GpSimd partition-broadcast and [1,S] row ops from the critical chain.
- Residual biases (bao + b_prev, bio + b_ln1) enter PSUM via K=1
  rank-1 matmuls appended to each accumulation group; the residual add
  is then a single scalar_tensor_tensor (g_prev * z_prev + psum).
- Activation-table swaps (exp/gelu/abs_rsqrt) are prefetched with dummy
  1-element activations during matmul phases, off the critical path.
- Paired-head softmax: both heads of a 128-feature block share one
  [P,1024] PSUM tile and a single fused Exp activation.
"""
import sys
import os

if "/opt/trn_rl_repo" not in sys.path:
    sys.path.insert(0, "/opt/trn_rl_repo")

import numpy as np
import ml_dtypes

import concourse.bass as bass
from concourse import bacc
import concourse.tile as tile
from concourse import mybir
from concourse.bass_utils import run_bass_kernel_spmd
from concourse.masks import make_identity

F32 = mybir.dt.float32
F32R = mybir.dt.float32r
BF16 = mybir.dt.bfloat16
FP16 = mybir.dt.float16
INT32 = mybir.dt.int32
AF = mybir.ActivationFunctionType
ALU = mybir.AluOpType

# Model dims (hardcoded per problem spec)
B, S, H, NH, L, F = 8, 512, 768, 12, 12, 3072
V, TV, PP = 21128, 2, 512
DH = H // NH            # 64
P = 128
HT = H // P             # 6
FT = F // P             # 24
ST = S // P             # 4
SH = S // 2             # 256  (sequence half)
EPS = 1e-12
NCORES = 8

NL = int(os.environ.get("BERT_KERNEL_LAYERS", str(L)))

_CACHE = {}


def _build():
    nc = bacc.Bacc("TRN2", target_bir_lowering=False, debug=False)

    # ---- DRAM I/O ----
    WQKVO = nc.dram_tensor("wqkvo", [NL, 4, P, HT, H], FP16, kind="ExternalInput")
    WI = nc.dram_tensor("wi", [NL, P, HT, F], FP16, kind="ExternalInput")
    WIO = nc.dram_tensor("wio", [NL, P, FT, H], FP16, kind="ExternalInput")
    PARAMS = nc.dram_tensor("params", [NL, P, 76], F32, kind="ExternalInput")
    CR = nc.dram_tensor("cr", [NL, 1, 2 * H], FP16, kind="ExternalInput")
    BVREP = nc.dram_tensor("bvrep", [NL, P, H], F32, kind="ExternalInput")
    TOK = nc.dram_tensor("tok", [V, H], F32, kind="ExternalInput")
    POSN = nc.dram_tensor("posn", [P, ST, H], F32, kind="ExternalInput")
    DTE = nc.dram_tensor("dte", [P, H], F32, kind="ExternalInput")
    IDS = nc.dram_tensor("ids", [P, ST], INT32, kind="ExternalInput")
    SEGF = nc.dram_tensor("segf", [P, ST], F32, kind="ExternalInput")
    MASKT = nc.dram_tensor("maskt", [P, ST], F32, kind="ExternalInput")
    OUT = nc.dram_tensor("out", [H, S], F32, kind="ExternalOutput")

    outv = OUT[:].rearrange("(ht p) s -> p ht s", p=P)

    DEBUG_TAPS = int(os.environ.get("BERT_DEBUG_TAPS", "0"))
    TAPS = ["z0", "q", "k", "ctx", "pre1", "z1", "pre2"]
    if DEBUG_TAPS:
        DBG = nc.dram_tensor("dbg", [len(TAPS), P, HT, S], F32,
                             kind="ExternalOutput")

    def hs(h):
        return slice(h * SH, (h + 1) * SH)

    from contextlib import ExitStack

    with tile.TileContext(nc) as tc:
        with ExitStack() as es:
            ec = es.enter_context
            cpool = ec(tc.tile_pool(name="const", bufs=1))
            apool = ec(tc.tile_pool(name="act", bufs=1))     # big per-layer activations
            rpool = ec(tc.tile_pool(name="res", bufs=2))     # z (bf16) ping-pong
            crpool = ec(tc.tile_pool(name="crp", bufs=2))    # bias rows per layer
            scr = ec(tc.tile_pool(name="scr", bufs=6))       # [P,256] scratch
            t1p = ec(tc.tile_pool(name="t1p", bufs=7))       # LN t1 tiles
            rsdpool = ec(tc.tile_pool(name="rsd", bufs=2))   # rstd|mq per half
            rows = ec(tc.tile_pool(name="rows", bufs=2))     # [1,S] rows
            bcp = ec(tc.tile_pool(name="bcast", bufs=4))     # attn bcast temps
            bvpool = ec(tc.tile_pool(name="bv", bufs=1))
            eppool = ec(tc.tile_pool(name="ep", bufs=2))     # exp tiles
            pp = ec(tc.tile_pool(name="pp", bufs=2, space="PSUM"))    # 2 banks
            psc = ec(tc.tile_pool(name="psc", bufs=2, space="PSUM"))  # 4 banks
            pcx = ec(tc.tile_pool(name="pcx", bufs=2, space="PSUM"))  # 2 banks
            # ---- persistent constants ----
            params = cpool.tile([P, NL, 76], F32, tag="params")
            nc.sync.dma_start(params[:], PARAMS[:].rearrange("l p c -> p l c"))
            maskt = cpool.tile([P, ST], F32, tag="maskt")
            nc.sync.dma_start(maskt[:], MASKT[:])
            ones_b = cpool.tile([P, P], BF16, tag="ones")
            nc.any.memset(ones_b[:], 1.0)
            ones_bh = cpool.tile([P, P], BF16, tag="onesbh")
            nc.any.memset(ones_bh[:], 1.0 / H)
            onesrow = cpool.tile([1, S], FP16, tag="onesrow")
            nc.any.memset(onesrow[:], 1.0)
            onesf = cpool.tile([1, P], F32, tag="onesf")
            nc.any.memset(onesf[:], 1.0)
            epsc = cpool.tile([P, 1], F32, tag="epsc")
            nc.any.memset(epsc[:], EPS)
            dscr = cpool.tile([1, 1], F32, tag="dscr")
            nc.any.memset(dscr[:], 0.25)

            # ---- persistent activations ----
            qTb = apool.tile([P, HT, S], FP16, tag="qTb")
            kTb = apool.tile([P, HT, S], FP16, tag="kTb")
            ctxTb = apool.tile([P, HT, S], FP16, tag="ctxTb")
            v_aug = apool.tile([P, ST, NH * (DH + 1)], BF16, tag="v_aug")
            hTb = apool.tile([P, FT, S], FP16, tag="hTb")
            preF = apool.tile([P, HT, S], FP16, tag="preF")

            # ones columns of v_aug (written once; per-layer V writes leave them)
            va_view = v_aug[:].rearrange("p st (h d) -> p st h d", d=DH + 1)
            nc.any.memset(va_view[:, :, :, DH : DH + 1], 1.0)

            # ============ embedding (scoped pool, released after) ============
            xzb = rpool.tile([P, HT, S], FP16, tag="resz")
            with tc.tile_pool(name="embp", bufs=3) as embp:
                posn = embp.tile([P, ST, H], F32, tag="posn", bufs=1)
                nc.sync.dma_start(posn[:], POSN[:])
                dte = embp.tile([P, H], F32, tag="dte", bufs=1)
                nc.sync.dma_start(dte[:], DTE[:])
                ids = embp.tile([P, ST], INT32, tag="ids", bufs=1)
                nc.sync.dma_start(ids[:], IDS[:])
                segf = embp.tile([P, ST], F32, tag="segf", bufs=1)
                nc.sync.dma_start(segf[:], SEGF[:])
                ident = embp.tile([P, P], F32, tag="ident", bufs=1)
                make_identity(nc, ident)

                for st in range(ST):
                    x0 = embp.tile([P, H], F32, tag="x0")
                    nc.gpsimd.indirect_dma_start(
                        out=x0[:],
                        out_offset=None,
                        in_=TOK[:],
                        in_offset=bass.IndirectOffsetOnAxis(
                            ap=ids[:, st : st + 1], axis=0
                        ),
                    )
                    # + (pos + type0) + seg*(type1-type0)
                    tseg = embp.tile([P, H], F32, tag="tseg")
                    nc.scalar.activation(
                        tseg[:], dte[:], AF.Copy, scale=segf[:, st : st + 1]
                    )
                    nc.vector.tensor_add(out=x0[:], in0=x0[:], in1=posn[:, st])
                    nc.gpsimd.tensor_tensor(x0[:], x0[:], tseg[:], ALU.add)
                    # LayerNorm along free dim (features); z only (g,b folded)
                    s1 = embp.tile([P, 1], F32, tag="s1")
                    nc.vector.reduce_sum(s1[:], x0[:], axis=mybir.AxisListType.X)
                    sqs = embp.tile([P, H], F32, tag="sqs")
                    ssq = embp.tile([P, 1], F32, tag="ssq")
                    nc.scalar.activation(sqs[:], x0[:], AF.Square, accum_out=ssq[:])
                    mean = embp.tile([P, 1], F32, tag="mean")
                    nc.any.tensor_scalar_mul(mean[:], s1[:], 1.0 / H)
                    msq = embp.tile([P, 1], F32, tag="msq")
                    nc.any.tensor_scalar_mul(msq[:], ssq[:], 1.0 / H)
                    var = embp.tile([P, 1], F32, tag="var")
                    nc.vector.tensor_tensor(var[:], mean[:], mean[:], ALU.mult)
                    nc.vector.tensor_tensor(var[:], msq[:], var[:], ALU.subtract)
                    rstd = embp.tile([P, 1], F32, tag="rstd")
                    lnv0 = embp.tile([P, 1], F32, tag="lnv0")
                    nc.scalar.activation(lnv0[:], var[:], AF.Ln, bias=epsc[:])
                    nc.scalar.activation(rstd[:], lnv0[:], AF.Exp, scale=-0.5)
                    negmr = embp.tile([P, 1], F32, tag="negmr")
                    nc.vector.tensor_tensor(negmr[:], mean[:], rstd[:], ALU.mult)
                    nc.any.tensor_scalar_mul(negmr[:], negmr[:], -1.0)
                    nc.scalar.activation(
                        x0[:], x0[:], AF.Identity, bias=negmr[:], scale=rstd[:]
                    )
                    # transpose into z layout (bf16)
                    for ht in range(HT):
                        pt = pp.tile([P, S], F32, tag="proj")
                        nc.tensor.transpose(
                            pt[:, :P], x0[:, P * ht : P * (ht + 1)], ident[:]
                        )
                        nc.scalar.copy(xzb[:, ht, P * st : P * (st + 1)], pt[:, :P])

            # ============ transformer layers ============

            def ln_half(pst, h, srcF, l, final):
                """Stats chain + apply for one sequence half.

                pst[:, :SH] = broadcast mean, pst[:, SH:] = broadcast meansq.
                Writes zdst bf16 (or the final DRAM output when `final`).
                Returns the z tile (None when final)."""
                pb = params[:, l, :]
                m2 = scr.tile([P, SH], F32, tag="s")
                nc.scalar.activation(m2[:], pst[:, 0:SH], AF.Square)
                varb = scr.tile([P, SH], F32, tag="s")
                nc.vector.tensor_tensor(varb[:], pst[:, SH:S], m2[:], ALU.subtract)
                rsd = rsdpool.tile([P, S], FP16, tag="rsd")
                nc.scalar.activation(
                    rsd[:, 0:SH], varb[:], AF.Abs_reciprocal_sqrt, bias=epsc[:]
                )
                return rsd

            def ln_sub_mean(pst, h, srcF, t1s):
                # phase A: t1 = x - mean_b (Vector: PSUM read); starts
                # before rstd is ready
                for kt in range(HT):
                    t1 = t1p.tile([P, SH], FP16, tag="t")
                    nc.vector.tensor_tensor(
                        t1[:], srcF[:, kt, hs(h)], pst[:, 0:SH], ALU.subtract
                    )
                    t1s.append(t1)

            def ln_apply(rsd, h, t1s, zdst, l, final):
                pb = params[:, l, :]
                for kt in range(HT):
                    eng = nc.vector
                    if not final:
                        eng.tensor_tensor(
                            zdst[:, kt, hs(h)], t1s[kt][:], rsd[:, 0:SH],
                            ALU.mult,
                        )
                    else:
                        of = scr.tile([P, SH], F32, tag="s")
                        eng.tensor_tensor(of[:], t1s[kt][:], rsd[:, 0:SH],
                                          ALU.mult)
                        nc.scalar.activation(
                            of[:], of[:], AF.Identity,
                            bias=pb[:, 30 + kt : 31 + kt],
                            scale=pb[:, 24 + kt : 25 + kt],
                        )
                        nc.sync.dma_start(outv[:, kt, hs(h)], of[:])

            def tap(name, src, l):
                if not DEBUG_TAPS or l != 0:
                    return
                ti = TAPS.index(name)
                for kt in range(HT):
                    if src.dtype == F32:
                        nc.sync.dma_start(DBG[ti][:, kt], src[:, kt])
                    else:
                        dc = scr.tile([P, S], F32, tag="dbg", bufs=2)
                        nc.scalar.activation(dc[:], src[:, kt], AF.Identity)
                        nc.sync.dma_start(DBG[ti][:, kt], dc[:])

            tap("z0", xzb, 0)

            wqpool = ec(tc.tile_pool(name="wq", bufs=3))
            wfpool = ec(tc.tile_pool(name="wf", bufs=2))
            wopool = ec(tc.tile_pool(name="wo", bufs=1))
            if True:
                for l in range(NL):
                    pb = params[:, l, :]
                    crows = crpool.tile([1, 2 * H], FP16, tag="crows")
                    nc.sync.dma_start(crows[:], CR[l])

                    # ---- Q, K projections (transposed out; consume xzb) ----
                    scope_qk = nc.named_scope(f"qk_{l}"); scope_qk.__enter__()
                    for pi, dst in ((0, qTb), (1, kTb)):
                        w = wqpool.tile([P, HT, H], FP16, tag="wqk")
                        nc.sync.dma_start(w[:], WQKVO[l, pi])
                        for h in range(2):
                            for nt in range(HT):
                                ps = pp.tile([P, S], F32, tag="proj")
                                for kt in range(HT):
                                    nc.tensor.matmul(
                                        ps[:, :SH],
                                        w[:, kt, P * nt : P * (nt + 1)],
                                        xzb[:, kt, hs(h)],
                                        start=(kt == 0), stop=(kt == HT - 1),
                                    )
                                c = 6 * pi + nt
                                nc.vector.tensor_scalar_add(
                                    dst[:, nt, hs(h)], ps[:, :SH],
                                    pb[:, c : c + 1],
                                )

                    scope_qk.__exit__(None, None, None)
                    tap("q", qTb, l)
                    tap("k", kTb, l)
                    scope_v = nc.named_scope(f"v_{l}"); scope_v.__enter__()
                    # ---- V projection (normal layout, augmented buffer) ----
                    nc.scalar.activation(dscr[:], qTb[0:1, 0, 0:1], AF.Exp)
                    wv = wqpool.tile([P, HT, H], FP16, tag="wqk")
                    nc.sync.dma_start(wv[:], WQKVO[l, 2])
                    bvr = bvpool.tile([P, H], F32, tag="bvr")
                    nc.sync.dma_start(bvr[:], BVREP[l])
                    for st in range(ST):
                        for half in range(2):
                            ps = pp.tile([P, S], F32, tag="proj")
                            for kt in range(HT):
                                nc.tensor.matmul(
                                    ps[:, :384],
                                    xzb[:, kt, P * st : P * (st + 1)],
                                    wv[:, kt, 384 * half : 384 * (half + 1)],
                                    start=(kt == 0), stop=(kt == HT - 1),
                                )
                            dst3 = va_view[:, st, 6 * half : 6 * (half + 1), 0:DH]
                            src3 = ps[:, :384].rearrange("p (h d) -> p h d", d=DH)
                            bv3 = bvr[:, 384 * half : 384 * (half + 1)].rearrange(
                                "p (h d) -> p h d", d=DH
                            )
                            nc.vector.tensor_tensor(dst3, src3, bv3, ALU.add)

                    scope_v.__exit__(None, None, None)
                    scope_at = nc.named_scope(f"attn_{l}"); scope_at.__enter__()
                    # ---- attention (head pairs share ht; fused pair Exp) ----
                    for ht in range(HT):
                        exq = eppool.tile([P, ST, 2 * S], BF16, tag="exq")
                        for kti in range(ST):
                            pshh = psc.tile([P, 2 * S], F32, tag="sc")
                            for sub in range(2):
                                base = 64 * sub
                                nc.tensor.matmul(
                                    pshh[:, S * sub : S * (sub + 1)],
                                    kTb[base : base + DH, ht, P * kti : P * (kti + 1)],
                                    qTb[base : base + DH, ht, :],
                                    start=True, stop=True,
                                )
                            nc.scalar.activation(
                                exq[:, kti], pshh[:], AF.Exp,
                                scale=0.125, bias=maskt[:, kti : kti + 1],
                            )
                        for sub in range(2):
                            hh = 2 * ht + sub
                            base = 64 * sub
                            ppc = pcx.tile([DH + 1, S], F32, tag="cx")
                            for kti in range(ST):
                                nc.tensor.matmul(
                                    ppc[:],
                                    v_aug[:, kti, (DH + 1) * hh : (DH + 1) * (hh + 1)],
                                    exq[:, kti, S * sub : S * (sub + 1)],
                                    start=(kti == 0), stop=(kti == ST - 1),
                                )
                            # 1/denominator on the PSUM row directly, then
                            # broadcast via K=1 matmul on the (idle) PE
                            srow0 = rows.tile([1, S], F32, tag="srow0")
                            nc.scalar.copy(srow0[:], ppc[DH : DH + 1, :])
                            rec = rows.tile([1, S], F32, tag="srow0")
                            nc.vector.reciprocal_approx_fast(rec[:], srow0[:])
                            rb = bcp.tile([DH, S], F32, tag="b")
                            nc.gpsimd.partition_broadcast(rb[:], rec[:])
                            nc.vector.tensor_tensor(
                                ctxTb[base : base + DH, ht, :], ppc[:DH, :],
                                rb[:], ALU.mult,
                            )

                    scope_at.__exit__(None, None, None)
                    tap("ctx", ctxTb, l)
                    scope_ao = nc.named_scope(f"ao_{l}"); scope_ao.__enter__()
                    # ---- attention output + residual + LN1 (S-split) ----
                    wao = wqpool.tile([P, HT, H], FP16, tag="wqk")
                    nc.sync.dma_start(wao[:], WQKVO[l, 3])
                    azb = rpool.tile([P, HT, S], FP16, tag="resz")
                    for h in range(2):
                        pst = pcx.tile([P, S], F32, tag="cx")
                        for nt in range(HT):
                            ps = pp.tile([P, S], F32, tag="proj")
                            # (bao + b_prev) rank-1 seed first: no deps, so
                            # it fills the attention tail and keeps PE warm
                            nc.tensor.matmul(
                                ps[:, :SH],
                                crows[0:1, P * nt : P * (nt + 1)],
                                onesrow[0:1, :SH],
                                start=True, stop=False,
                            )
                            for kt in range(HT):
                                nc.tensor.matmul(
                                    ps[:, :SH],
                                    wao[:, kt, P * nt : P * (nt + 1)],
                                    ctxTb[:, kt, hs(h)],
                                    start=False, stop=(kt == HT - 1),
                                )
                            # pre1 = ps + g_prev * z_prev
                            nc.vector.scalar_tensor_tensor(
                                preF[:, nt, hs(h)], xzb[:, nt, hs(h)],
                                pb[:, 12 + nt : 13 + nt], ps[:, :SH],
                                ALU.mult, ALU.add,
                            )
                            if h == 0 and nt == 0:
                                # dep-pinned abs_rsqrt table prefetch
                                nc.scalar.activation(
                                    dscr[:], preF[0:1, 0, 0:1],
                                    AF.Abs_reciprocal_sqrt,
                                )
                            # stats (broadcast form)
                            # single accumulation group per bank:
                            # sc2 = [x/H | x*x/H] -> [mean_b | msq_b]
                            sc2 = scr.tile([P, S], BF16, tag="s")
                            nc.scalar.activation(
                                sc2[:, 0:SH], preF[:, nt, hs(h)], AF.Copy,
                                scale=1.0 / H,
                            )
                            nc.vector.tensor_tensor(
                                sc2[:, SH:S], sc2[:, 0:SH],
                                preF[:, nt, hs(h)], ALU.mult,
                            )
                            nc.tensor.matmul(
                                pst[:, 0:S], ones_b[:], sc2[:],
                                start=(nt == 0), stop=(nt == HT - 1),
                            )
                        t1s = []
                        ln_sub_mean(pst, h, preF, t1s)
                        rsd = ln_half(pst, h, preF, l, False)
                        ln_apply(rsd, h, t1s, azb, l, False)
                    tap("pre1", preF, l)
                    tap("z1", azb, l)
                    scope_ao.__exit__(None, None, None)

                    scope_f1 = nc.named_scope(f"ffn1_{l}"); scope_f1.__enter__()
                    # ---- FFN intermediate (gelu) ----
                    for quarter in range(4):
                        wih = wfpool.tile([P, HT, F // 4], FP16, tag="wi")
                        nc.sync.dma_start(
                            wih[:],
                            WI[l][:, :, (F // 4) * quarter : (F // 4) * (quarter + 1)],
                        )
                        for h in range(2):
                            for ntl in range(6):
                                nt = 6 * quarter + ntl
                                ps = pp.tile([P, S], F32, tag="proj")
                                for kt in range(HT):
                                    nc.tensor.matmul(
                                        ps[:, :SH],
                                        wih[:, kt, P * ntl : P * (ntl + 1)],
                                        azb[:, kt, hs(h)],
                                        start=(kt == 0), stop=(kt == HT - 1),
                                    )
                                nc.scalar.activation(
                                    hTb[:, nt, hs(h)], ps[:, :SH], AF.Gelu,
                                    bias=pb[:, 48 + nt : 49 + nt],
                                )

                    scope_f1.__exit__(None, None, None)
                    scope_f2 = nc.named_scope(f"ffn2_{l}"); scope_f2.__enter__()
                    # ---- FFN output + residual + LN2 (S-split) ----
                    wio = wopool.tile([P, FT, H], FP16, tag="wio")
                    nc.sync.dma_start(wio[:], WIO[l])
                    final = l == NL - 1
                    if not final:
                        xzb = rpool.tile([P, HT, S], FP16, tag="resz")
                    for h in range(2):
                        pst = pcx.tile([P, S], F32, tag="cx")
                        for nt in range(HT):
                            ps = pp.tile([P, S], F32, tag="proj")
                            nc.tensor.matmul(
                                ps[:, :SH],
                                crows[0:1, H + P * nt : H + P * (nt + 1)],
                                onesrow[0:1, :SH],
                                start=True, stop=False,
                            )
                            for kt in range(FT):
                                nc.tensor.matmul(
                                    ps[:, :SH],
                                    wio[:, kt, P * nt : P * (nt + 1)],
                                    hTb[:, kt, hs(h)],
                                    start=False, stop=(kt == FT - 1),
                                )
                            # pre2 = ps + g_ln1 * z_attn
                            nc.vector.scalar_tensor_tensor(
                                preF[:, nt, hs(h)], azb[:, nt, hs(h)],
                                pb[:, 18 + nt : 19 + nt], ps[:, :SH],
                                ALU.mult, ALU.add,
                            )
                            if h == 0 and nt == 0:
                                nc.scalar.activation(
                                    dscr[:], preF[0:1, 0, 0:1],
                                    AF.Abs_reciprocal_sqrt,
                                )
                            # single accumulation group per bank:
                            # sc2 = [x/H | x*x/H] -> [mean_b | msq_b]
                            sc2 = scr.tile([P, S], BF16, tag="s")
                            nc.scalar.activation(
                                sc2[:, 0:SH], preF[:, nt, hs(h)], AF.Copy,
                                scale=1.0 / H,
                            )
                            nc.vector.tensor_tensor(
                                sc2[:, SH:S], sc2[:, 0:SH],
                                preF[:, nt, hs(h)], ALU.mult,
                            )
                            nc.tensor.matmul(
                                pst[:, 0:S], ones_b[:], sc2[:],
                                start=(nt == 0), stop=(nt == HT - 1),
                            )
                        t1s = []
                        ln_sub_mean(pst, h, preF, t1s)
                        rsd = ln_half(pst, h, preF, l, final)
                        ln_apply(rsd, h, t1s, None if final else xzb, l, final)
                    tap("pre2", preF, l)
                    scope_f2.__exit__(None, None, None)

    nc.compile()
    return nc


def _r6(v):
    return np.ascontiguousarray(v.reshape(6, P).T)


def _prep_shared(inputs):
    bf = ml_dtypes.bfloat16
    fh = np.float16
    f32 = np.float32
    emb_g = np.asarray(inputs["emb_g"], f32)
    emb_b = np.asarray(inputs["emb_b"], f32)
    ln1_g = np.asarray(inputs["ln1_g"], f32)
    ln1_b = np.asarray(inputs["ln1_b"], f32)
    ln2_g = np.asarray(inputs["ln2_g"], f32)
    ln2_b = np.asarray(inputs["ln2_b"], f32)

    wqkvo = np.empty((NL, 4, P, HT, H), dtype=fh)
    wi = np.empty((NL, P, HT, F), dtype=fh)
    wio = np.empty((NL, P, FT, H), dtype=fh)
    params = np.zeros((NL, P, 76), dtype=f32)
    cr = np.zeros((NL, 1, 2 * H), dtype=fh)
    bvrep = np.empty((NL, P, H), dtype=f32)

    def tposed(w, kt, dim_out):
        # [H, dim_out] -> [P, kt, dim_out]
        return w.reshape(kt, P, dim_out).transpose(1, 0, 2)

    for l in range(NL):
        gprev = emb_g if l == 0 else ln2_g[l - 1]
        bprev = emb_b if l == 0 else ln2_b[l - 1]
        for pi, name, bname in ((0, "Wq", "bq"), (1, "Wk", "bk"), (2, "Wv", "bv")):
            w = np.asarray(inputs[name][l], f32)
            b = np.asarray(inputs[bname][l], f32)
            wf = w * gprev[:, None]
            beff = b + bprev @ w
            wqkvo[l, pi] = tposed(wf, HT, H).astype(fh)
            if pi == 0:
                params[l, :, 0:6] = _r6(beff)
            elif pi == 1:
                params[l, :, 6:12] = _r6(beff)
            else:
                bvrep[l] = np.broadcast_to(beff, (P, H))
        wao = np.asarray(inputs["Wao"][l], f32)
        wqkvo[l, 3] = tposed(wao, HT, H).astype(fh)
        wi_l = np.asarray(inputs["Wi"][l], f32)
        wi[l] = tposed(wi_l * ln1_g[l][:, None], HT, F).astype(fh)
        bi_eff = np.asarray(inputs["bi"][l], f32) + ln1_b[l] @ wi_l
        wio[l] = tposed(np.asarray(inputs["Wio"][l], f32), FT, H).astype(fh)

        params[l, :, 12:18] = _r6(gprev)
        params[l, :, 18:24] = _r6(ln1_g[l])
        params[l, :, 24:30] = _r6(ln2_g[l])
        params[l, :, 30:36] = _r6(ln2_b[l])
        params[l, :, 48:72] = bi_eff.reshape(FT, P).T
        cr[l, 0, 0:H] = (np.asarray(inputs["bao"][l], f32) + bprev).astype(fh)
        cr[l, 0, H : 2 * H] = (
            np.asarray(inputs["bio"][l], f32) + ln1_b[l]
        ).astype(fh)

    tok = np.ascontiguousarray(np.asarray(inputs["tok_emb"], f32))
    te = np.asarray(inputs["type_emb"], f32)
    posn = np.ascontiguousarray(
        (np.asarray(inputs["pos_emb"], f32)[:S] + te[0])
        .reshape(ST, P, H).transpose(1, 0, 2)
    )
    dte = np.ascontiguousarray(np.broadcast_to(te[1] - te[0], (P, H)))
    return {
        "wqkvo": wqkvo, "wi": wi, "wio": wio, "params": params,
        "cr": cr, "bvrep": bvrep, "tok": tok, "posn": posn, "dte": dte,
    }


def kernel(**inputs):
    if "nc" not in _CACHE:
        _CACHE["nc"] = _build()
    nc = _CACHE["nc"]

    shared = _prep_shared(inputs)
    ids_full = np.asarray(inputs["input_ids"], np.int32)
    seg_full = np.asarray(inputs["segment_ids"], np.int32)
    mask_full = np.asarray(inputs["attention_mask"], np.float32)

    in_maps = []
    for c in range(NCORES):
        m = dict(shared)
        m["ids"] = np.ascontiguousarray(ids_full[c].reshape(ST, P).T)
        m["segf"] = np.ascontiguousarray(
            seg_full[c].astype(np.float32).reshape(ST, P).T
        )
        mrow = (1.0 - mask_full[c, 0, 0]) * -10000.0
        m["maskt"] = np.ascontiguousarray(mrow.reshape(ST, P).T)
        in_maps.append(m)

    res = run_bass_kernel_spmd(nc, in_maps, core_ids=list(range(NCORES)))
    out = np.empty((B, S, H), dtype=np.float32)
    for c in range(NCORES):
        out[c] = res.results[c]["out"].T
    return out
